# revision 1
# baseline (speedup 1.0000x reference)
"""DPA+SSM block kernel for 8 Trainium2 NeuronCores.

Sharding: data-parallel over the sequence (T=4096 -> 8 x 512 own tokens);
each core also receives a 256-token halo of the raw input before its own
range.  The attention window is 256, so the halo covers every key a core
needs; the SSM recurrence decay |A| < 0.1 makes state influence from before
the halo underflow fp32 entirely, so a zero-initialized scan warm-started
over the halo is exact.  No cross-core communication.

Layout: activations are feature-major [D, T] on the device (host transposes
in/out).  All linears run as W-chunk-stationary x activation-moving fp32r
matmuls.  LayerNorm statistics use an all-ones stationary matmul (partition
reduction, replicated over partitions).  The SSM scan is one exact DVE
tensor_tensor_scan.  V is produced token-major with an interleaved ones
column per head so each PV matmul also emits the softmax denominator.
"""

import sys

try:
    import concourse.bass as bass  # noqa: F401
except Exception:
    sys.path.insert(0, "/opt/trn_rl_repo")

import numpy as np

import concourse.bass as bass  # noqa: F401
import concourse.mybir as mybir
from concourse import bacc, bass_utils
from concourse.tile import TileContext

F32 = mybir.dt.float32
F32R = mybir.dt.float32r

D = 1024
S = 128
H = 16
DH = 64
C = 256          # attention window / block size
T = 4096
NCORES = 8
TOWN = T // NCORES        # 512 own tokens per core
HALO = C                  # 256 halo tokens
TLOC = TOWN + HALO        # 768 local rows per core
EPS = 1e-5

# bias-pack column layout
BC_A = 0
BC_QKV = 1       # 24 cols
BC_GATE = 25     # 8
BC_DRIVE = 33    # 1
BC_O = 34        # 8
BC_B1 = 42       # 32
BC_B2 = 74       # 8
NBC = 82

T_TILES = [(0, 512), (512, 256)]
OWN0 = HALO


def _r(ap):
    """Identity: matmul operands are natively float32r-typed."""
    return ap


def build_program(reps=1):
    nc = bacc.Bacc("TRN2", target_bir_lowering=False, debug=False)
    dt = F32
    d_xT = nc.dram_tensor("xT", [D, TLOC], F32R, kind="ExternalInput").ap()
    d_mask0 = nc.dram_tensor("mask0", [128, 4 * C], dt, kind="ExternalInput").ap()
    d_mask1 = nc.dram_tensor("mask1", [128, 4 * C], dt, kind="ExternalInput").ap()
    d_bias = nc.dram_tensor("biaspack", [128, NBC], dt, kind="ExternalInput").ap()
    d_vbias = nc.dram_tensor("vbias", [1, D], F32R, kind="ExternalInput").ap()
    d_wqkv = nc.dram_tensor("wqkv", [D, 3 * D], F32R, kind="ExternalInput").ap()
    d_wgate = nc.dram_tensor("wgate", [D, D], F32R, kind="ExternalInput").ap()
    d_wdrive = nc.dram_tensor("wdrive", [D, S], F32R, kind="ExternalInput").ap()
    d_wo = nc.dram_tensor("wo", [D, D], F32R, kind="ExternalInput").ap()
    d_cw = nc.dram_tensor("cw", [S, D], F32R, kind="ExternalInput").ap()
    d_w1 = nc.dram_tensor("w1", [D, 4 * D], F32R, kind="ExternalInput").ap()
    d_w2 = nc.dram_tensor("w2", [4 * D, D], F32R, kind="ExternalInput").ap()
    d_out = nc.dram_tensor("outT", [D, TOWN], dt, kind="ExternalOutput").ap()

    AF = mybir.ActivationFunctionType
    OP = mybir.AluOpType

    def persist(pool, shape, tag, dtype=F32):
        return pool.tile(shape, dtype, tag=tag, name=tag, bufs=1)

    def _tsub(o, a, b):
        nc.vector.tensor_sub(o, a, b)

    with TileContext(nc) as tc:
        for _rep in range(reps):
            with tc.tile_pool(name="const", bufs=1) as const, \
                 tc.tile_pool(name="xmid", bufs=1) as xm_pool:
                biasp = persist(const, [128, NBC], "biasp")
                nc.sync.dma_start(biasp[:], d_bias[:])
                mask0 = persist(const, [128, 4 * C], "mask0")
                nc.sync.dma_start(mask0[:], d_mask0[:])
                mask1 = persist(const, [128, 4 * C], "mask1")
                nc.sync.dma_start(mask1[:], d_mask1[:])
                ones_f = persist(const, [128, 128], "ones_f")
                nc.vector.memset(ones_f[:], 1.0 / D)
                ones_ln = persist(const, [128, 128], "ones_ln", F32R)
                nc.scalar.activation(ones_ln[:], ones_f[:], AF.Copy, bias=0.0)
                vbrow = persist(const, [1, D], "vbrow", F32R)
                nc.sync.dma_start(vbrow[:], d_vbias[:])
                vbias = persist(const, [128, D], "vbias")
                a_full = persist(const, [128, TLOC], "a_full")
                nc.vector.memset(a_full[:], 1.0)
                nc.vector.tensor_scalar_mul(a_full[:], a_full[:],
                                            biasp[:, BC_A:BC_A + 1])
                eps_col = persist(const, [128, 1], "eps_col")
                nc.vector.memset(eps_col[:], EPS)
                onesr_f = persist(const, [1, 128], "onesr_f")
                nc.vector.memset(onesr_f[:], 1.0)
                ones_row = persist(const, [1, 128], "ones_row", F32R)
                nc.scalar.activation(ones_row[:], onesr_f[:], AF.Copy, bias=0.0)

                def bias_col(idx):
                    return biasp[:, idx:idx + 1]

                def layernorm(xs, tfree, stats, spsum, out_pool, tagp):
                    mu = persist(stats, [128, tfree], f"mu{tagp}")
                    var = persist(stats, [128, tfree], f"var{tagp}")
                    rstd = persist(stats, [128, tfree], f"rstd{tagp}")
                    ttl = [(t0, tw) for (t0, tw) in T_TILES if t0 < tfree]
                    for t0, tw in ttl:
                        ps_mu = spsum.tile([128, 512], F32, tag="ln_mu")
                        ps_sq = spsum.tile([128, 512], F32, tag="ln_sq")
                        for c in range(8):
                            nc.tensor.matmul(ps_mu[:, :tw], _r(ones_ln[:]),
                                             _r(xs[c][:, t0:t0 + tw]),
                                             start=(c == 0), stop=(c == 7))
                        nc.vector.tensor_copy(mu[:, t0:t0 + tw], ps_mu[:, :tw])
                        for c in range(8):
                            sq = stats.tile([128, 512], F32R, tag="sq", bufs=3)
                            nc.scalar.activation(sq[:, :tw],
                                                 xs[c][:, t0:t0 + tw],
                                                 AF.Square)
                            nc.tensor.matmul(ps_sq[:, :tw], _r(ones_ln[:]),
                                             _r(sq[:, :tw]),
                                             start=(c == 0), stop=(c == 7))
                        nc.vector.tensor_mul(var[:, t0:t0 + tw], mu[:, t0:t0 + tw],
                                             mu[:, t0:t0 + tw])
                        _tsub(var[:, t0:t0 + tw], ps_sq[:, :tw], var[:, t0:t0 + tw])
                        nc.scalar.activation(rstd[:, t0:t0 + tw],
                                             var[:, t0:t0 + tw], AF.Sqrt,
                                             bias=eps_col[:])
                        nc.vector.reciprocal(rstd[:, t0:t0 + tw],
                                             rstd[:, t0:t0 + tw])
                    outs = []
                    for c in range(8):
                        o = persist(out_pool, [128, tfree], f"{tagp}{c}", F32R)
                        _tsub(o[:], xs[c][:], mu[:])
                        nc.vector.tensor_mul(o[:], o[:], rstd[:])
                        outs.append(o)
                    return outs

                # ====== phase group A: LN1, projections, attention, fusion =====
                with tc.tile_pool(name="act1", bufs=1) as act1:
                    gs = [persist(act1, [128, TOWN], f"g{c}") for c in range(8)]
                    attns = [persist(act1, [128, TOWN], f"at{c}", F32R) for c in range(8)]
                    driveT = persist(act1, [128, TLOC], "driveT")
                    states = persist(act1, [128, TLOC], "states", F32R)

                    with tc.tile_pool(name="act0", bufs=1) as act0:
                        kts = [persist(act0, [128, TLOC], f"k{c}", F32R) for c in range(8)]
                        vts = [persist(act0, [128, 16 * 65], f"v{c}", F32R)
                               for c in range(6)]
                        qts = [persist(act0, [128, TOWN], f"q{c}", F32R) for c in range(8)]
                        vones = persist(act0, [128, 16], "vones")
                        nc.vector.memset(vones[:], 1.0)
                        for tt in range(6):
                            vr = vts[tt][:].rearrange("p (h e) -> p h e", e=65)
                            nc.scalar.activation(
                                vr[:, :, 64:65],
                                vones[:].rearrange("p (a b) -> p a b", b=1),
                                AF.Copy, bias=0.0)

                        xn_pool_outer = act0
                        with tc.tile_pool(name="xT", bufs=1) as xtp, \
                             tc.tile_pool(name="lnst", bufs=1) as lnst, \
                             tc.tile_pool(name="lnpsum", bufs=2,
                                          space="PSUM") as lnp:
                            xts = [persist(xtp, [128, TLOC], f"x{c}", F32R)
                                   for c in range(8)]
                            for c in range(8):
                                nc.sync.dma_start(
                                    xts[c][:], d_xT[c * 128:(c + 1) * 128, :])
                            xns = layernorm(xts, TLOC, lnst, lnp,
                                            xn_pool_outer, "xn")

                        # ---- projections -----------------------------------
                        with tc.tile_pool(name="wlin", bufs=12) as wp, \
                             tc.tile_pool(name="linpsum", bufs=6,
                                          space="PSUM") as psum:

                            for half2 in range(2):
                                psb = psum.tile([128, 512], F32, tag="lin")
                                nc.tensor.matmul(
                                    psb[:], ones_row[:],
                                    vbrow[:, half2 * 512:(half2 + 1) * 512],
                                    start=True, stop=True)
                                nc.vector.tensor_copy(
                                    vbias[:, half2 * 512:(half2 + 1) * 512],
                                    psb[:])

                            def load_w(dram, kc, m0, mw):
                                w = wp.tile([128, 512], F32R, tag="w")
                                nc.sync.dma_start(
                                    w[:, :mw],
                                    dram[kc * 128:(kc + 1) * 128, m0:m0 + mw])
                                return w

                            # K chunks (qkv cols 1024..2048), all 768 rows
                            for mg in range(2):
                                ws = [load_w(d_wqkv, kc, D + mg * 512, 512)
                                      for kc in range(8)]
                                for j in range(4):
                                    mc = mg * 4 + j
                                    for t0, tw in T_TILES:
                                        ps = psum.tile([128, 512], F32, tag="lin")
                                        for kc in range(8):
                                            nc.tensor.matmul(
                                                ps[:, :tw],
                                                _r(ws[kc][:, j * 128:(j + 1) * 128]),
                                                _r(xns[kc][:, t0:t0 + tw]),
                                                start=(kc == 0), stop=(kc == 7))
                                        nc.vector.tensor_scalar(
                                            kts[mc][:, t0:t0 + tw], ps[:, :tw],
                                            bias_col(BC_QKV + 8 + mc), None, OP.add)
                            # V chunks (qkv cols 2048..3072), token-major
                            for vc in range(2):
                                ws = [load_w(d_wqkv, kc, 2 * D + vc * 512, 512)
                                      for kc in range(8)]
                                for tt in range(6):
                                    t0 = tt * 128
                                    ps = psum.tile([128, 512], F32, tag="lin")
                                    for kc in range(8):
                                        nc.tensor.matmul(
                                            ps[:], _r(xns[kc][:, t0:t0 + 128]),
                                            _r(ws[kc][:]),
                                            start=(kc == 0), stop=(kc == 7))
                                    vr = vts[tt][:].rearrange(
                                        "p (h e) -> p h e", e=65)
                                    dst = vr[:, vc * 8:(vc + 1) * 8, 0:64]
                                    nc.vector.tensor_copy(
                                        dst,
                                        ps[:].rearrange("p (h e) -> p h e",
                                                        e=64))
                                    nc.vector.tensor_add(
                                        dst, dst,
                                        vbias[:, vc * 512:(vc + 1) * 512].rearrange(
                                            "p (h e) -> p h e", e=64))
                            # Q chunks (qkv cols 0..1024), own rows only
                            for mg in range(2):
                                ws = [load_w(d_wqkv, kc, mg * 512, 512)
                                      for kc in range(8)]
                                for j in range(4):
                                    mc = mg * 4 + j
                                    ps = psum.tile([128, 512], F32, tag="lin")
                                    for kc in range(8):
                                        nc.tensor.matmul(
                                            ps[:],
                                            _r(ws[kc][:, j * 128:(j + 1) * 128]),
                                            _r(xns[kc][:, OWN0:OWN0 + TOWN]),
                                            start=(kc == 0), stop=(kc == 7))
                                    nc.vector.tensor_scalar(
                                        qts[mc][:], ps[:], bias_col(BC_QKV + mc),
                                        None, OP.add)
                            # gate (sigmoid), own rows only
                            for mg in range(2):
                                ws = [load_w(d_wgate, kc, mg * 512, 512)
                                      for kc in range(8)]
                                for j in range(4):
                                    mc = mg * 4 + j
                                    ps = psum.tile([128, 512], F32, tag="lin")
                                    for kc in range(8):
                                        nc.tensor.matmul(
                                            ps[:],
                                            _r(ws[kc][:, j * 128:(j + 1) * 128]),
                                            _r(xns[kc][:, OWN0:OWN0 + TOWN]),
                                            start=(kc == 0), stop=(kc == 7))
                                    nc.scalar.activation(gs[mc][:], ps[:],
                                                         AF.Sigmoid,
                                                         bias=bias_col(BC_GATE + mc))
                            # drive, all 768 rows
                            ws = [load_w(d_wdrive, kc, 0, 128) for kc in range(8)]
                            for t0, tw in T_TILES:
                                ps = psum.tile([128, 512], F32, tag="lin")
                                for kc in range(8):
                                    nc.tensor.matmul(
                                        ps[:, :tw], _r(ws[kc][:, :128]),
                                        _r(xns[kc][:, t0:t0 + tw]),
                                        start=(kc == 0), stop=(kc == 7))
                                nc.vector.tensor_scalar(
                                    driveT[:, t0:t0 + tw], ps[:, :tw],
                                    bias_col(BC_DRIVE), None, OP.add)

                        # ---- windowed attention ----------------------------
                        with tc.tile_pool(name="apsum", bufs=3,
                                          space="PSUM") as apsum, \
                             tc.tile_pool(name="ptp", bufs=12) as ptp, \
                             tc.tile_pool(name="rp", bufs=3) as rp:
                            for b in range(2):
                                mask = mask0 if b == 0 else mask1
                                for h in range(H):
                                    cch = h // 2
                                    half = (h % 2) * 64
                                    pts = []
                                    for kc in range(4):
                                        st = apsum.tile([128, C], F32, tag="st")
                                        k0 = C * b + 128 * kc
                                        nc.tensor.matmul(
                                            st[:],
                                            _r(kts[cch][half:half + 64,
                                                        k0:k0 + 128]),
                                            _r(qts[cch][half:half + 64,
                                                        C * b:C * (b + 1)]),
                                            start=True, stop=True)
                                        pt = ptp.tile([128, C], F32R, tag="pt")
                                        nc.scalar.activation(
                                            pt[:], st[:], AF.Exp,
                                            scale=float(1.0 / np.sqrt(DH)))
                                        nc.vector.tensor_mul(
                                            pt[:], pt[:],
                                            mask[:, kc * C:(kc + 1) * C])
                                        pts.append(pt)
                                    po = apsum.tile([65, C], F32, tag="po",
                                                    bufs=3)
                                    for kc in range(4):
                                        nc.tensor.matmul(
                                            po[:],
                                            _r(vts[2 * b + kc][:,
                                               h * 65:(h + 1) * 65]),
                                            _r(pts[kc][:]),
                                            start=(kc == 0), stop=(kc == 3))
                                    rrow = rp.tile([1, C], F32R, tag="rr")
                                    with nc.allow_low_precision(
                                            reason="f32r rounding of softmax "
                                            "denominators is benign"):
                                        nc.vector.reciprocal(rrow[:],
                                                             po[64:65, :])
                                    rb = apsum.tile([64, C], F32, tag="rb",
                                                    bufs=2)
                                    nc.tensor.matmul(rb[:],
                                                     ones_row[:, :64],
                                                     rrow[:],
                                                     start=True, stop=True)
                                    rs = rp.tile([64, C], F32, tag="r64")
                                    nc.vector.tensor_copy(rs[:], rb[:])
                                    nc.vector.tensor_mul(
                                        attns[cch][half:half + 64,
                                                   C * b:C * (b + 1)],
                                        po[0:64, :], rs[:])
                    # act0 closed: xn/kt/v/qt freed

                    # ---- SSM scan + projections + fusion -------------------
                    nc.vector.tensor_tensor_scan(states[:], a_full[:], driveT[:],
                                                 0.0, OP.mult, OP.add)
                    with tc.tile_pool(name="fus", bufs=1) as fus, \
                         tc.tile_pool(name="wfus", bufs=10) as wf, \
                         tc.tile_pool(name="spsum", bufs=4, space="PSUM") as sp:
                        ys = [persist(fus, [128, TOWN], f"y{c}") for c in range(8)]
                        xos = [persist(fus, [128, TOWN], f"xo{c}", F32R) for c in range(8)]
                        for c in range(8):
                            nc.sync.dma_start(
                                xos[c][:],
                                d_xT[c * 128:(c + 1) * 128, OWN0:OWN0 + TOWN])
                        wcs = []
                        for mg in range(2):
                            w = wf.tile([128, 512], F32R, tag="w")
                            nc.sync.dma_start(w[:], d_cw[:, mg * 512:(mg + 1) * 512])
                            wcs.append(w)
                        for mc in range(8):
                            ps = sp.tile([128, 512], F32, tag="s")
                            nc.tensor.matmul(
                                ps[:],
                                _r(wcs[mc // 4][:, (mc % 4) * 128:(mc % 4 + 1) * 128]),
                                _r(states[:, OWN0:OWN0 + TOWN]),
                                start=True, stop=True)
                            nc.vector.tensor_copy(ys[mc][:], ps[:])
                        xms = [persist(xm_pool, [128, TOWN], f"xm{c}", F32R)
                               for c in range(8)]
                        for mg in range(2):
                            wos = [load_w_pool(wf, nc, d_wo, kc, mg * 512, 512)
                                   for kc in range(8)]
                            for j in range(4):
                                mc = mg * 4 + j
                                ps = sp.tile([128, 512], F32, tag="s")
                                for kc in range(8):
                                    nc.tensor.matmul(
                                        ps[:],
                                        _r(wos[kc][:, j * 128:(j + 1) * 128]),
                                        _r(attns[kc][:]),
                                        start=(kc == 0), stop=(kc == 7))
                                xm = xms[mc]
                                nc.vector.scalar_tensor_tensor(
                                    xm[:], ps[:], bias_col(BC_O + mc), ys[mc][:],
                                    op0=OP.add, op1=OP.subtract)
                                nc.vector.tensor_mul(xm[:], xm[:], gs[mc][:])
                                nc.vector.tensor_add(xm[:], xm[:], ys[mc][:])
                                nc.vector.tensor_add(xm[:], xm[:], xos[mc][:])
                # act1 closed: g/attn/drive/states freed

                # ====== phase group B: LN2 + MLP ===============================
                with tc.tile_pool(name="xn2p", bufs=1) as xn2p:
                    with tc.tile_pool(name="lnst2", bufs=1) as lnst2, \
                         tc.tile_pool(name="ln2psum", bufs=2, space="PSUM") as lnp2:
                        xn2s = layernorm(xms, TOWN, lnst2, lnp2, xn2p, "h")
                    with tc.tile_pool(name="hTp", bufs=1) as hTp, \
                         tc.tile_pool(name="wmlp", bufs=12) as wm:
                        hts = [persist(hTp, [128, TOWN], f"ht{c}", F32R)
                               for c in range(32)]
                        with tc.tile_pool(name="m1psum", bufs=6,
                                          space="PSUM") as mp1:
                            for mg in range(8):
                                ws = [load_w_pool(wm, nc, d_w1, kc, mg * 512, 512)
                                      for kc in range(8)]
                                for j in range(4):
                                    mc = mg * 4 + j
                                    ps = mp1.tile([128, 512], F32, tag="m")
                                    for kc in range(8):
                                        nc.tensor.matmul(
                                            ps[:],
                                            _r(ws[kc][:, j * 128:(j + 1) * 128]),
                                            _r(xn2s[kc][:]),
                                            start=(kc == 0), stop=(kc == 7))
                                    nc.scalar.activation(
                                        hts[mc][:], ps[:], AF.Gelu,
                                        bias=bias_col(BC_B1 + mc))
                        with tc.tile_pool(name="m2psum", bufs=1,
                                          space="PSUM") as mp2, \
                             tc.tile_pool(name="outp", bufs=3) as outp:
                            pss = [mp2.tile([128, 512], F32, tag=f"o{mc}",
                                            name=f"o{mc}", bufs=1)
                                   for mc in range(8)]
                            for kc in range(32):
                                w2r = wm.tile([128, 1024], F32R, tag="w2", bufs=3)
                                nc.sync.dma_start(
                                    w2r[:], d_w2[kc * 128:(kc + 1) * 128, :])
                                for mc in range(8):
                                    nc.tensor.matmul(
                                        pss[mc][:],
                                        _r(w2r[:, mc * 128:(mc + 1) * 128]),
                                        _r(hts[kc][:]),
                                        start=(kc == 0), stop=(kc == 31))
                            for mc in range(8):
                                oc = outp.tile([128, TOWN], F32, tag="oc")
                                nc.vector.scalar_tensor_tensor(
                                    oc[:], pss[mc][:], bias_col(BC_B2 + mc),
                                    xms[mc][:], op0=OP.add, op1=OP.add)
                                nc.sync.dma_start(
                                    d_out[mc * 128:(mc + 1) * 128, :], oc[:])

    nc.compile()
    return nc


def load_w_pool(pool, nc, dram, kc, m0, mw):
    w = pool.tile([128, 512], F32R, tag="w")
    nc.sync.dma_start(w[:, :mw], dram[kc * 128:(kc + 1) * 128, m0:m0 + mw])
    return w


def _make_masks():
    qi = np.arange(C)[:, None]
    kk = np.arange(2 * C)[None, :]
    band = (kk > qi) & (kk <= qi + C)
    first = band & (kk >= C)

    def pack(m):                       # [C, 2C] -> [128, 4*C] k-chunk-major
        mt = m.T.astype(np.float32)    # [2C, C]
        return np.ascontiguousarray(
            mt.reshape(4, 128, C).transpose(1, 0, 2).reshape(128, 4 * C))

    return pack(first), pack(band)


def _prep_inputs(x, ln1_g, ln1_b, ln2_g, ln2_b, W_qkv, W_O, b_O, W_ug, b_ug,
                 B_w, A, C_w, mlp_W1, mlp_b1, mlp_W2, mlp_b2):
    f = np.float32
    g1 = np.asarray(ln1_g, f)
    b1 = np.asarray(ln1_b, f)
    W_qkv = np.asarray(W_qkv, f)
    W_qkv_e = g1[:, None] * W_qkv
    b_qkv_e = b1 @ W_qkv
    W_ug = np.asarray(W_ug, f)
    B_w = np.asarray(B_w, f)
    b_ug = np.asarray(b_ug, f)
    W_drive_raw = B_w + W_ug[:, :S]
    W_drive_e = g1[:, None] * W_drive_raw
    b_drive_e = b1 @ W_drive_raw + b_ug[:S]
    W_gate_e = g1[:, None] * W_ug[:, S:]
    b_gate_e = b1 @ W_ug[:, S:] + b_ug[S:]
    g2 = np.asarray(ln2_g, f)
    b2l = np.asarray(ln2_b, f)
    mlp_W1 = np.asarray(mlp_W1, f)
    W1_e = g2[:, None] * mlp_W1
    b1_e = b2l @ mlp_W1 + np.asarray(mlp_b1, f)

    biaspack = np.zeros((128, NBC), f)
    biaspack[:, BC_A] = np.asarray(A, f)
    biaspack[:, BC_QKV:BC_QKV + 24] = b_qkv_e.reshape(24, 128).T
    biaspack[:, BC_GATE:BC_GATE + 8] = b_gate_e.reshape(8, 128).T
    biaspack[:, BC_DRIVE] = b_drive_e
    biaspack[:, BC_O:BC_O + 8] = np.asarray(b_O, f).reshape(8, 128).T
    biaspack[:, BC_B1:BC_B1 + 32] = b1_e.reshape(32, 128).T
    biaspack[:, BC_B2:BC_B2 + 8] = np.asarray(mlp_b2, f).reshape(8, 128).T
    vbias = np.ascontiguousarray(b_qkv_e[2 * D:].reshape(1, D))

    m_first, m_band = _make_masks()
    xTfull = np.ascontiguousarray(np.asarray(x, f)[0].T)

    shared = {
        "biaspack": biaspack, "vbias": vbias,
        "wqkv": np.ascontiguousarray(W_qkv_e),
        "wgate": np.ascontiguousarray(W_gate_e),
        "wdrive": np.ascontiguousarray(W_drive_e),
        "wo": np.ascontiguousarray(np.asarray(W_O, f)),
        "cw": np.ascontiguousarray(np.asarray(C_w, f)),
        "w1": np.ascontiguousarray(W1_e),
        "w2": np.ascontiguousarray(np.asarray(mlp_W2, f)),
        "mask1": m_band,
    }
    in_maps = []
    for i in range(NCORES):
        t0 = i * TOWN
        xT = np.zeros((D, TLOC), f)
        lo = max(0, t0 - HALO)
        xT[:, HALO - (t0 - lo):HALO] = xTfull[:, lo:t0]
        xT[:, HALO:] = xTfull[:, t0:t0 + TOWN]
        m0 = m_first if i == 0 else m_band
        in_maps.append({**shared, "xT": np.ascontiguousarray(xT), "mask0": m0})
    return in_maps


_CACHED_NC = None


def get_nc():
    global _CACHED_NC
    if _CACHED_NC is None:
        _CACHED_NC = build_program()
    return _CACHED_NC


def kernel(**inputs):
    nc = get_nc()
    in_maps = _prep_inputs(**inputs)
    res = bass_utils.run_bass_kernel_spmd(nc, in_maps,
                                          core_ids=list(range(NCORES)))
    out = np.empty((1, T, D), np.float32)
    for i in range(NCORES):
        out[0, i * TOWN:(i + 1) * TOWN, :] = res.results[i]["outT"].T
    return out



# revision 43
# speedup vs baseline: 1.1863x; 1.1863x over previous
"""DPA+SSM block kernel for 8 Trainium2 NeuronCores.

Sharding: data-parallel over the sequence (T=4096 -> 8 x 512 own tokens);
each core also receives a 256-token halo of the raw input before its own
range.  The attention window is 256, so the halo covers every key a core
needs; the SSM recurrence decay |A| < 0.1 makes state influence from before
the halo underflow fp32 entirely, so a zero-initialized scan warm-started
over the halo is exact.  No cross-core communication.

Layout: activations are feature-major [D, T] on the device (host transposes
in/out).  Weights and activations are bf16 (halves HBM traffic and doubles
DVE throughput; matmul cost on TRN2 is 1 row/cycle for bf16 and for fp32r
with free dim >= 256, so precision is the only trade — final rel err stays
~1e-3, well inside the 2e-2 gate).  LayerNorm statistics use an all-ones
stationary matmul (partition reduction, replicated over partitions), kept
in bf16 with f32 PSUM accumulation.  The SSM scan is one exact f32 DVE
tensor_tensor_scan.  V carries an interleaved ones column per head so each
PV matmul also emits the softmax denominator.  Elementwise epilogues are
spread across DVE / Pool(gpsimd) / Activation so no single engine gates
the Tensor engine; activation-table switches (sqrt/sigmoid/exp/gelu) are
prefetched with 1-element dummy ops during idle windows.
"""

import sys

try:
    import concourse.bass as bass  # noqa: F401
except Exception:
    sys.path.insert(0, "/opt/trn_rl_repo")

import numpy as np

import concourse.bass as bass  # noqa: F401
import concourse.mybir as mybir
from concourse import bacc, bass_utils
from concourse.tile import TileContext

F32 = mybir.dt.float32
F32R = mybir.dt.float32r
BF16 = mybir.dt.bfloat16
NPBF16 = mybir.dt.np(mybir.dt.bfloat16)

D = 1024
S = 128
H = 16
DH = 64
C = 256          # attention window / block size
T = 4096
NCORES = 8
TOWN = T // NCORES        # 512 own tokens per core
HALO = C                  # 256 halo tokens
TLOC = TOWN + HALO        # 768 local rows per core
EPS = 1e-5

# bias-pack column layout (f32 per-partition scalars)
BC_A = 0
BC_QKV = 1       # 24 cols
BC_GATE = 25     # 8
BC_DRIVE = 33    # 1
BC_O = 34        # 8
BC_B1 = 42       # 32
BC_B2 = 74       # 8
NBC = 82

T_TILES = [(0, 512), (512, 256)]
OWN0 = HALO


def _r(ap):
    return ap


def build_program(reps=1):
    nc = bacc.Bacc("TRN2", target_bir_lowering=False, debug=False)
    d_xT = nc.dram_tensor("xT", [D, TLOC], BF16, kind="ExternalInput").ap()
    d_mask0 = nc.dram_tensor("mask0", [128, 4 * C], BF16, kind="ExternalInput").ap()
    d_mask1 = nc.dram_tensor("mask1", [128, 4 * C], BF16, kind="ExternalInput").ap()
    d_bias = nc.dram_tensor("biaspack", [128, NBC], F32, kind="ExternalInput").ap()
    d_vbias = nc.dram_tensor("vbias", [1, D], F32R, kind="ExternalInput").ap()
    d_wqkv = nc.dram_tensor("wqkv", [D, 3 * D], BF16, kind="ExternalInput").ap()
    d_wgate = nc.dram_tensor("wgate", [D, D], BF16, kind="ExternalInput").ap()
    d_wdrive = nc.dram_tensor("wdrive", [D, S], BF16, kind="ExternalInput").ap()
    d_wo = nc.dram_tensor("wo", [D, D], BF16, kind="ExternalInput").ap()
    d_cw = nc.dram_tensor("cw", [S, D], F32R, kind="ExternalInput").ap()
    d_w1 = nc.dram_tensor("w1", [D, 4 * D], BF16, kind="ExternalInput").ap()
    d_w2 = nc.dram_tensor("w2", [4 * D, D], BF16, kind="ExternalInput").ap()
    d_out = nc.dram_tensor("outT", [D, TOWN], BF16, kind="ExternalOutput").ap()

    AF = mybir.ActivationFunctionType
    OP = mybir.AluOpType

    def persist(pool, shape, tag, dtype=BF16):
        return pool.tile(shape, dtype, tag=tag, name=tag, bufs=1)

    with TileContext(nc) as tc:
        for _rep in range(reps):
            with tc.tile_pool(name="const", bufs=1) as const, \
                 tc.tile_pool(name="xmid", bufs=1) as xm_pool:
                # const tiles: allocate now; DMAs for bias/masks/vbias are
                # issued AFTER the x-chunk DMAs (those gate the first matmul)
                biasp = persist(const, [128, NBC], "biasp", F32)
                mask0 = persist(const, [128, 4 * C], "mask0")
                mask1 = persist(const, [128, 4 * C], "mask1")
                vbrow = persist(const, [1, D], "vbrow", F32R)
                a_full = persist(const, [128, TLOC], "a_full", F32)
                eps_col = persist(const, [128, 1], "eps_col", F32)
                nc.vector.memset(eps_col[:], EPS)
                dummy = persist(const, [1, 1], "dummy", F32)

                def act_prefetch(func):
                    # 1-element activation to pull the act table in during an
                    # idle window instead of on the critical path
                    nc.scalar.activation(dummy[:], eps_col[0:1, :], func)

                # sqrt table first so the LN1 rstd sqrt needs no reload
                act_prefetch(AF.Sqrt)
                ones_ln = persist(const, [128, 128], "ones_ln", BF16)
                nc.vector.memset(ones_ln[:], 1.0 / D)
                onesr_f = persist(const, [1, 128], "onesr_f", F32)
                nc.vector.memset(onesr_f[:], 1.0)
                ones_row = persist(const, [1, 128], "ones_row", F32R)
                nc.scalar.activation(ones_row[:], onesr_f[:], AF.Copy,
                                     bias=0.0)

                def bias_col(idx):
                    return biasp[:, idx:idx + 1]

                def layernorm(xs, tfree, stats, spsum, out_pool, tagp,
                              sq_pool_cs=(), norm_pool_cs=(2, 6)):
                    """Stats via ones-matmul; normalize split DVE/Pool, bf16.

                    Normalization is emitted per (t-tile, chunk) so the first
                    t-tile's outputs exist before the second tile's stats are
                    done — downstream matmul chains start ~5us earlier.
                    """
                    mu = persist(stats, [128, tfree], f"mu{tagp}")
                    rstd = persist(stats, [128, tfree], f"rstd{tagp}")
                    var = persist(stats, [128, tfree], f"var{tagp}", F32)
                    outs = [persist(out_pool, [128, tfree], f"{tagp}{c}")
                            for c in range(8)]
                    ttl = [(t0, tw) for (t0, tw) in T_TILES if t0 < tfree]
                    for t0, tw in ttl:
                        ps_mu = spsum.tile([128, 512], F32, tag="ln_mu")
                        ps_sq = spsum.tile([128, 512], F32, tag="ln_sq")
                        for c in range(8):
                            nc.tensor.matmul(ps_mu[:, :tw], _r(ones_ln[:]),
                                             _r(xs[c][:, t0:t0 + tw]),
                                             start=(c == 0), stop=(c == 7))
                        nc.vector.tensor_copy(mu[:, t0:t0 + tw], ps_mu[:, :tw])
                        for c in range(8):
                            sq = stats.tile([128, 512], BF16, tag="sq", bufs=3)
                            # square on DVE/Pool, not Act: keeps the Act
                            # queue clear of table-set churn
                            seng = (nc.gpsimd if (t0 > 0 or c in sq_pool_cs)
                                    else nc.vector)
                            seng.tensor_mul(sq[:, :tw], xs[c][:, t0:t0 + tw],
                                            xs[c][:, t0:t0 + tw])
                            nc.tensor.matmul(ps_sq[:, :tw], _r(ones_ln[:]),
                                             _r(sq[:, :tw]),
                                             start=(c == 0), stop=(c == 7))
                        nc.vector.tensor_mul(var[:, t0:t0 + tw],
                                             mu[:, t0:t0 + tw],
                                             mu[:, t0:t0 + tw])
                        nc.vector.tensor_sub(var[:, t0:t0 + tw],
                                             ps_sq[:, :tw], var[:, t0:t0 + tw])
                        nc.scalar.activation(var[:, t0:t0 + tw],
                                             var[:, t0:t0 + tw], AF.Sqrt,
                                             bias=eps_col[:])
                        with nc.allow_low_precision(
                                reason="bf16 rstd: 0.4% scale error on "
                                "normalized activations is benign"):
                            nc.vector.reciprocal(rstd[:, t0:t0 + tw],
                                                 var[:, t0:t0 + tw])
                        for c in range(8):
                            o = outs[c]
                            eng = nc.gpsimd if c in norm_pool_cs else nc.vector
                            eng.tensor_sub(o[:, t0:t0 + tw],
                                           xs[c][:, t0:t0 + tw],
                                           mu[:, t0:t0 + tw])
                            eng.tensor_mul(o[:, t0:t0 + tw],
                                           o[:, t0:t0 + tw],
                                           rstd[:, t0:t0 + tw])
                    return outs

                # ====== phase group A: LN1, projections, attention, fusion ====
                with tc.tile_pool(name="act1", bufs=1) as act1:
                    gs = [persist(act1, [128, TOWN], f"g{c}") for c in range(8)]
                    attns = [persist(act1, [128, TOWN], f"at{c}")
                             for c in range(8)]
                    driveT = persist(act1, [128, TLOC], "driveT", F32)
                    states = persist(act1, [128, TLOC], "states", F32R)
                    # x resident for the whole phase: LN1 input + residual.
                    # One wide tile, TWO DMAs total (HWDGE descriptor
                    # generation is a fixed 625ns per DMA instruction, so
                    # fewer/wider transfers beat per-chunk loads)
                    xpairs = [persist(act1, [128, 2 * TLOC], f"xp{i}")
                              for i in range(4)]
                    wc = persist(act1, [128, 1024], "wc", F32R)
                    wos = [persist(act1, [128, 4096], f"wo{mg}")
                           for mg in range(2)]
                    xts = [xpairs[c // 2][:, (c % 2) * TLOC:
                                          (c % 2 + 1) * TLOC]
                           for c in range(8)]
                    xos = [xpairs[c // 2][:, (c % 2) * TLOC + OWN0:
                                          (c % 2 + 1) * TLOC]
                           for c in range(8)]

                    with tc.tile_pool(name="act0", bufs=1) as act0:
                        kts = [persist(act0, [128, TLOC], f"k{c}") for c in range(8)]
                        vts = [persist(act0, [128, 16 * 65], f"v{c}")
                               for c in range(6)]
                        qts = [persist(act0, [128, TOWN], f"q{c}") for c in range(8)]
                        for tt in range(6):
                            vr = vts[tt][:].rearrange("p (h e) -> p h e", e=65)
                            nc.gpsimd.memset(vr[:, :, 64:65], 1.0)

                        with tc.tile_pool(name="xnp", bufs=1) as xnp, \
                             tc.tile_pool(name="lnst", bufs=1) as lnst:
                            for i in range(4):
                                nc.sync.dma_start(
                                    xpairs[i][:],
                                    d_xT[i * 256:(i + 1) * 256, :].rearrange(
                                        "(c p) t -> p c t", p=128))
                            # const DMAs after the x chunks they'd delay
                            # (masks wait further: not needed until attention)
                            nc.sync.dma_start(biasp[:], d_bias[:])
                            nc.sync.dma_start(vbrow[:], d_vbias[:])
                            with tc.tile_pool(name="lnpsum", bufs=2,
                                              space="PSUM") as lnp:
                                xns = layernorm(xts, TLOC, lnst, lnp, xnp, "xn")

                            # ---- projections -------------------------------
                            # one DMA per 8-chunk weight group: HWDGE costs a
                            # fixed 625ns per DMA instruction, so group loads
                            # through a strided (c p) m -> p (c m) pattern
                            with tc.tile_pool(name="wlin", bufs=9) as wp, \
                                 tc.tile_pool(name="linpsum", bufs=5,
                                              space="PSUM") as psum:

                                def load_wg(dram, m0, mw, tag="w"):
                                    w = wp.tile([128, 8 * mw], BF16, tag=tag)
                                    nc.sync.dma_start(
                                        w[:],
                                        dram[0:1024, m0:m0 + mw].rearrange(
                                            "(c p) m -> p c m", p=128))
                                    return w

                                # All chains split by LN1 t-tile: every chain
                                # that only needs tile-0 xn (cols 0:512) is
                                # emitted before any chain touching tile 1,
                                # so the in-order PE queue streams behind the
                                # LN1 normalize instead of stalling on it.
                                wgates = [load_wg(d_wgate, mg * 512, 512)
                                          for mg in range(2)]
                                wks = [load_wg(d_wqkv, D + mg * 512, 512)
                                       for mg in range(2)]
                                wvs = [load_wg(d_wqkv, 2 * D + vc * 512, 512)
                                       for vc in range(2)]
                                wqs = [load_wg(d_wqkv, mg * 512, 512)
                                       for mg in range(2)]
                                wd = load_wg(d_wdrive, 0, 128, tag="wd")

                                def gate_q(t0, tw):
                                    # gate first: its sigmoids are the only
                                    # Act work here, so Act reaches the
                                    # exp-table prefetch early
                                    for wgrp, outs, bc0, act in (
                                            (wgates, gs, BC_GATE, True),
                                            (wqs, qts, BC_QKV, False)):
                                        for mg in range(2):
                                            for j in range(4):
                                                mc = mg * 4 + j
                                                ps = psum.tile([128, 512], F32,
                                                               tag="lin")
                                                for kc in range(8):
                                                    w0 = kc * 512 + j * 128
                                                    nc.tensor.matmul(
                                                        ps[:, :tw],
                                                        _r(wgrp[mg][:,
                                                           w0:w0 + 128]),
                                                        _r(xns[kc][:,
                                                           t0:t0 + tw]),
                                                        start=(kc == 0),
                                                        stop=(kc == 7))
                                                o0 = t0 - OWN0
                                                if act:
                                                    nc.scalar.activation(
                                                        outs[mc][:, o0:o0 + tw],
                                                        ps[:, :tw], AF.Sigmoid,
                                                        bias=bias_col(bc0 + mc))
                                                else:
                                                    nc.vector.tensor_scalar(
                                                        outs[mc][:, o0:o0 + tw],
                                                        ps[:, :tw],
                                                        bias_col(bc0 + mc),
                                                        None, OP.add)

                                def k_drive(t0, tw):
                                    for mg in range(2):
                                        for j in range(4):
                                            mc = mg * 4 + j
                                            ps = psum.tile([128, 512], F32,
                                                           tag="lin")
                                            for kc in range(8):
                                                w0 = kc * 512 + j * 128
                                                nc.tensor.matmul(
                                                    ps[:, :tw],
                                                    _r(wks[mg][:, w0:w0 + 128]),
                                                    _r(xns[kc][:, t0:t0 + tw]),
                                                    start=(kc == 0),
                                                    stop=(kc == 7))
                                            nc.vector.tensor_scalar(
                                                kts[mc][:, t0:t0 + tw],
                                                ps[:, :tw],
                                                bias_col(BC_QKV + 8 + mc),
                                                None, OP.add)
                                    ps = psum.tile([128, 512], F32, tag="lin")
                                    for kc in range(8):
                                        nc.tensor.matmul(
                                            ps[:, :tw],
                                            _r(wd[:, kc * 128:(kc + 1) * 128]),
                                            _r(xns[kc][:, t0:t0 + tw]),
                                            start=(kc == 0), stop=(kc == 7))
                                    nc.vector.tensor_scalar(
                                        driveT[:, t0:t0 + tw], ps[:, :tw],
                                        bias_col(BC_DRIVE), None, OP.add)

                                def v_block(tts):
                                    # V bias folded into the accumulation as
                                    # a rank-1 ones x vbrow matmul
                                    for vc in range(2):
                                        for tt in tts:
                                            t0 = tt * 128
                                            ps = psum.tile([128, 512], F32,
                                                           tag="lin")
                                            for kc in range(8):
                                                nc.tensor.matmul(
                                                    ps[:],
                                                    _r(xns[kc][:, t0:t0 + 128]),
                                                    _r(wvs[vc][:, kc * 512:
                                                               (kc + 1) * 512]),
                                                    start=(kc == 0), stop=False)
                                            nc.tensor.matmul(
                                                ps[:], ones_row[:, 0:128],
                                                vbrow[:,
                                                      vc * 512:(vc + 1) * 512],
                                                start=False, stop=True)
                                            vr = vts[tt][:].rearrange(
                                                "p (h e) -> p h e", e=65)
                                            nc.scalar.activation(
                                                vr[:, vc * 8:(vc + 1) * 8,
                                                   0:64],
                                                ps[:].rearrange(
                                                    "p (h e) -> p h e", e=64),
                                                AF.Copy, bias=0.0)

                                # tile-0-only chains
                                gate_q(OWN0, 512 - OWN0)
                                k_drive(0, 512)
                                v_block(range(4))
                                # tile-1 chains
                                gate_q(512, TLOC - 512)
                                act_prefetch(AF.Exp)
                                k_drive(512, TLOC - 512)
                                v_block((4, 5))
                        # xnp closed: xn freed before attention

                        # masks overlap late projections (x stays resident,
                        # no residual re-load needed)
                        nc.sync.dma_start(mask0[:], d_mask0[:])
                        nc.sync.dma_start(mask1[:], d_mask1[:])
                        # fusion + first MLP weights stream during attention
                        nc.sync.dma_start(wc[:], d_cw[:])
                        for mg in range(2):
                            nc.sync.dma_start(
                                wos[mg][:],
                                d_wo[0:1024,
                                     mg * 512:mg * 512 + 512].rearrange(
                                    "(c p) m -> p c m", p=128))
                        w1pre = persist(xm_pool, [128, 4096], "w1g0")
                        nc.sync.dma_start(
                            w1pre[:],
                            d_w1[0:1024, 0:512].rearrange(
                                "(c p) m -> p c m", p=128))
                        # scan coefficient, off the critical path on Pool
                        nc.gpsimd.memset(a_full[:], 1.0)
                        nc.gpsimd.tensor_scalar_mul(a_full[:], a_full[:],
                                                    biasp[:, BC_A:BC_A + 1])
                        # SSM scan emitted BEFORE attention: driveT is ready,
                        # and queueing it here keeps it off the back of the
                        # attention-heavy DVE queue, so states exist the
                        # moment the WO/Cw matmuls want them
                        nc.vector.tensor_tensor_scan(states[:], a_full[:],
                                                     driveT[:], 0.0,
                                                     OP.mult, OP.add)

                        # ---- windowed attention ----------------------------
                        # processed per head PAIR (both heads of one cch):
                        # one [2,128] selector matmul broadcasts both heads'
                        # softmax reciprocals at once, one Act copy stages
                        # them in SBUF (HW: DVE may read only one PSUM
                        # operand, Pool none)
                        with tc.tile_pool(name="apsum", bufs=2,
                                          space="PSUM") as apsum, \
                             tc.tile_pool(name="posum", bufs=3,
                                          space="PSUM") as posum, \
                             tc.tile_pool(name="rbsum", bufs=1,
                                          space="PSUM") as rbsum, \
                             tc.tile_pool(name="ptp", bufs=3) as ptp, \
                             tc.tile_pool(name="rp", bufs=4) as rp:
                            # 0/1 selector rows: rb2 = selA^T@rrA + selB^T@rrB
                            # broadcasts both heads' reciprocals in one
                            # [128,C] accumulation with a partition-0 dst
                            selA = persist(rp, [1, 128], "selA")
                            nc.gpsimd.memset(selA[:], 0.0)
                            nc.gpsimd.memset(selA[0:1, 0:64], 1.0)
                            selB = persist(rp, [1, 128], "selB")
                            nc.gpsimd.memset(selB[:], 0.0)
                            nc.gpsimd.memset(selB[0:1, 64:128], 1.0)
                            for b in range(2):
                                mask = mask0 if b == 0 else mask1
                                for cch in range(8):
                                    pos = []
                                    rrs = []
                                    for hi in range(2):
                                        h = 2 * cch + hi
                                        half = hi * 64
                                        st4 = apsum.tile([128, 4 * C], F32,
                                                         tag="st4")
                                        for kc in range(4):
                                            k0 = C * b + 128 * kc
                                            nc.tensor.matmul(
                                                st4[:, kc * C:(kc + 1) * C],
                                                _r(kts[cch][half:half + 64,
                                                            k0:k0 + 128]),
                                                _r(qts[cch][half:half + 64,
                                                            C * b:C * (b + 1)]),
                                                start=True, stop=True)
                                        pt4 = ptp.tile([128, 4 * C], BF16,
                                                       tag="pt4")
                                        nc.scalar.activation(
                                            pt4[:], st4[:], AF.Exp,
                                            scale=float(1.0 / np.sqrt(DH)))
                                        meng = nc.vector if hi == 0 \
                                            else nc.gpsimd
                                        meng.tensor_mul(pt4[:], pt4[:],
                                                        mask[:])
                                        po = posum.tile([65, C], F32,
                                                        tag="po")
                                        for kc in range(4):
                                            nc.tensor.matmul(
                                                po[:],
                                                _r(vts[2 * b + kc][:,
                                                   h * 65:(h + 1) * 65]),
                                                _r(pt4[:,
                                                   kc * C:(kc + 1) * C]),
                                                start=(kc == 0),
                                                stop=(kc == 3))
                                        pos.append(po)
                                        rr = rp.tile([1, C], BF16, tag="rr")
                                        with nc.allow_low_precision(
                                                reason="bf16 rounding of "
                                                "softmax denominators is "
                                                "benign"):
                                            nc.vector.reciprocal(
                                                rr[:], po[64:65, :])
                                        rrs.append(rr)
                                    rb2 = rbsum.tile([128, C], F32, tag="rb")
                                    for hi, sel in ((0, selA), (1, selB)):
                                        nc.tensor.matmul(
                                            rb2[:], sel[:], rrs[hi][:],
                                            start=(hi == 0), stop=(hi == 1))
                                    rs2 = rp.tile([128, C], BF16, tag="rs")
                                    nc.scalar.activation(rs2[:], rb2[:],
                                                         AF.Copy, bias=0.0)
                                    for hi in range(2):
                                        half = hi * 64
                                        nc.vector.tensor_mul(
                                            attns[cch][half:half + 64,
                                                       C * b:C * (b + 1)],
                                            pos[hi][0:64, :],
                                            rs2[half:half + 64, :])
                    # act0 closed: k/v/q freed

                    # ---- SSM output + WO + fusion --------------------------
                    act_prefetch(AF.Sqrt)  # LN2 table, during WO window
                    with tc.tile_pool(name="fus", bufs=1) as fus, \
                         tc.tile_pool(name="wfus", bufs=10) as wf, \
                         tc.tile_pool(name="spsum", bufs=4, space="PSUM") as sp:
                        ys = [persist(fus, [128, TOWN], f"y{c}") for c in range(8)]
                        zs = [persist(fus, [128, TOWN], f"z{c}") for c in range(8)]
                        for mc in range(8):
                            ps = sp.tile([128, 512], F32, tag="s")
                            nc.tensor.matmul(
                                ps[:],
                                _r(wc[:, mc * 128:(mc + 1) * 128]),
                                _r(states[:, OWN0:OWN0 + TOWN]),
                                start=True, stop=True)
                            nc.scalar.activation(ys[mc][:], ps[:], AF.Copy,
                                                 bias=0.0)
                            eng = nc.gpsimd if mc % 2 else nc.vector
                            eng.tensor_add(zs[mc][:], ys[mc][:], xos[mc])
                        xms = [persist(xm_pool, [128, TOWN], f"xm{c}")
                               for c in range(8)]
                        for mg in range(2):
                            for j in range(4):
                                mc = mg * 4 + j
                                ps = sp.tile([128, 512], F32, tag="s")
                                for kc in range(8):
                                    w0 = kc * 512 + j * 128
                                    nc.tensor.matmul(
                                        ps[:],
                                        _r(wos[mg][:, w0:w0 + 128]),
                                        _r(attns[kc][:]),
                                        start=(kc == 0), stop=(kc == 7))
                                xm = xms[mc]
                                nc.vector.scalar_tensor_tensor(
                                    xm[:], ps[:], bias_col(BC_O + mc),
                                    ys[mc][:], op0=OP.add, op1=OP.subtract)
                                eng = nc.gpsimd if mc in (2, 5, 7) else nc.vector
                                eng.tensor_mul(xm[:], xm[:], gs[mc][:])
                                eng.tensor_add(xm[:], xm[:], zs[mc][:])
                # act1 closed: g/attn/drive/states/xos freed

                # ====== phase group B: LN2 + MLP ==============================
                with tc.tile_pool(name="xn2p", bufs=1) as xn2p, \
                     tc.tile_pool(name="wmlp", bufs=3) as wm, \
                     tc.tile_pool(name="w2p", bufs=1) as w2p:

                    def load_w1g(mg):
                        w = wm.tile([128, 4096], BF16, tag="w1")
                        nc.sync.dma_start(
                            w[:],
                            d_w1[0:1024, mg * 512:mg * 512 + 512].rearrange(
                                "(c p) m -> p c m", p=128))
                        return w

                    with tc.tile_pool(name="lnst2", bufs=1) as lnst2:
                        with tc.tile_pool(name="ln2psum", bufs=2,
                                          space="PSUM") as lnp2:
                            xn2s = layernorm(xms, TOWN, lnst2, lnp2, xn2p, "h",
                                             sq_pool_cs=(1, 4, 6),
                                             norm_pool_cs=(2, 5))
                        act_prefetch(AF.Gelu)
                    with tc.tile_pool(name="hTp", bufs=1) as hTp:
                        hts = [persist(hTp, [128, TOWN], f"ht{c}")
                               for c in range(32)]
                        with tc.tile_pool(name="m1psum", bufs=6,
                                          space="PSUM") as mp1:
                            for mg in range(8):
                                ws = w1pre if mg == 0 else load_w1g(mg)
                                for j in range(4):
                                    mc = mg * 4 + j
                                    ps = mp1.tile([128, 512], F32, tag="m")
                                    for kc in range(8):
                                        w0 = kc * 512 + j * 128
                                        nc.tensor.matmul(
                                            ps[:],
                                            _r(ws[:, w0:w0 + 128]),
                                            _r(xn2s[kc][:]),
                                            start=(kc == 0), stop=(kc == 7))
                                    nc.scalar.activation(
                                        hts[mc][:], ps[:], AF.Gelu,
                                        bias=bias_col(BC_B1 + mc))
                        # all of W2 resident (64KB/partition, bf16), ONE DMA
                        # issued while MLP1 computes; mc-outer chains below
                        # let each output's epilogue+store overlap the next
                        w2all = persist(w2p, [128, 32 * 1024], "w2all")
                        nc.sync.dma_start(
                            w2all[:],
                            d_w2[0:4096, 0:1024].rearrange(
                                "(c p) m -> p c m", p=128))
                        with tc.tile_pool(name="m2psum", bufs=3,
                                          space="PSUM") as mp2, \
                             tc.tile_pool(name="outp", bufs=3) as outp:
                            for mc in range(8):
                                ps = mp2.tile([128, 512], F32, tag="o")
                                for kc in range(32):
                                    w0 = kc * 1024 + mc * 128
                                    nc.tensor.matmul(
                                        ps[:],
                                        _r(w2all[:, w0:w0 + 128]),
                                        _r(hts[kc][:]),
                                        start=(kc == 0), stop=(kc == 31))
                                oc = outp.tile([128, TOWN], BF16, tag="oc")
                                nc.vector.scalar_tensor_tensor(
                                    oc[:], ps[:], bias_col(BC_B2 + mc),
                                    xms[mc][:], op0=OP.add, op1=OP.add)
                                nc.sync.dma_start(
                                    d_out[mc * 128:(mc + 1) * 128, :], oc[:])

    nc.compile()
    return nc


def _make_masks():
    qi = np.arange(C)[:, None]
    kk = np.arange(2 * C)[None, :]
    band = (kk > qi) & (kk <= qi + C)
    first = band & (kk >= C)

    def pack(m):                       # [C, 2C] -> [128, 4*C] k-chunk-major
        mt = m.T.astype(NPBF16)        # [2C, C]
        return np.ascontiguousarray(
            mt.reshape(4, 128, C).transpose(1, 0, 2).reshape(128, 4 * C))

    return pack(first), pack(band)


def _prep_inputs(x, ln1_g, ln1_b, ln2_g, ln2_b, W_qkv, W_O, b_O, W_ug, b_ug,
                 B_w, A, C_w, mlp_W1, mlp_b1, mlp_W2, mlp_b2):
    f = np.float32
    g1 = np.asarray(ln1_g, f)
    b1 = np.asarray(ln1_b, f)
    W_qkv = np.asarray(W_qkv, f)
    W_qkv_e = g1[:, None] * W_qkv
    b_qkv_e = b1 @ W_qkv
    W_ug = np.asarray(W_ug, f)
    B_w = np.asarray(B_w, f)
    b_ug = np.asarray(b_ug, f)
    W_drive_raw = B_w + W_ug[:, :S]
    W_drive_e = g1[:, None] * W_drive_raw
    b_drive_e = b1 @ W_drive_raw + b_ug[:S]
    W_gate_e = g1[:, None] * W_ug[:, S:]
    b_gate_e = b1 @ W_ug[:, S:] + b_ug[S:]
    g2 = np.asarray(ln2_g, f)
    b2l = np.asarray(ln2_b, f)
    mlp_W1 = np.asarray(mlp_W1, f)
    W1_e = g2[:, None] * mlp_W1
    b1_e = b2l @ mlp_W1 + np.asarray(mlp_b1, f)

    biaspack = np.zeros((128, NBC), f)
    biaspack[:, BC_A] = np.asarray(A, f)
    biaspack[:, BC_QKV:BC_QKV + 24] = b_qkv_e.reshape(24, 128).T
    biaspack[:, BC_GATE:BC_GATE + 8] = b_gate_e.reshape(8, 128).T
    biaspack[:, BC_DRIVE] = b_drive_e
    biaspack[:, BC_O:BC_O + 8] = np.asarray(b_O, f).reshape(8, 128).T
    biaspack[:, BC_B1:BC_B1 + 32] = b1_e.reshape(32, 128).T
    biaspack[:, BC_B2:BC_B2 + 8] = np.asarray(mlp_b2, f).reshape(8, 128).T
    vbias = np.ascontiguousarray(b_qkv_e[2 * D:].reshape(1, D))

    m_first, m_band = _make_masks()
    xTfull = np.ascontiguousarray(np.asarray(x, f)[0].T.astype(NPBF16))

    def bf(a):
        return np.ascontiguousarray(np.asarray(a, f).astype(NPBF16))

    shared = {
        "biaspack": biaspack, "vbias": vbias,
        "wqkv": bf(W_qkv_e),
        "wgate": bf(W_gate_e),
        "wdrive": bf(W_drive_e),
        "wo": bf(W_O),
        "cw": np.ascontiguousarray(np.asarray(C_w, f)),
        "w1": bf(W1_e),
        "w2": bf(mlp_W2),
        "mask1": m_band,
    }
    in_maps = []
    for i in range(NCORES):
        t0 = i * TOWN
        xT = np.zeros((D, TLOC), NPBF16)
        lo = max(0, t0 - HALO)
        xT[:, HALO - (t0 - lo):HALO] = xTfull[:, lo:t0]
        xT[:, HALO:] = xTfull[:, t0:t0 + TOWN]
        m0 = m_first if i == 0 else m_band
        in_maps.append({**shared, "xT": np.ascontiguousarray(xT), "mask0": m0})
    return in_maps


_CACHED_NC = None


def get_nc():
    global _CACHED_NC
    if _CACHED_NC is None:
        _CACHED_NC = build_program()
    return _CACHED_NC


def kernel(**inputs):
    nc = get_nc()
    in_maps = _prep_inputs(**inputs)
    res = bass_utils.run_bass_kernel_spmd(nc, in_maps,
                                          core_ids=list(range(NCORES)))
    out = np.empty((1, T, D), np.float32)
    for i in range(NCORES):
        out[0, i * TOWN:(i + 1) * TOWN, :] = \
            res.results[i]["outT"].astype(np.float32).T
    return out


# revision 46
# speedup vs baseline: 1.4073x; 1.1862x over previous
"""DPA+SSM block kernel for 8 Trainium2 NeuronCores.

Sharding: data-parallel over the sequence (T=4096 -> 8 x 512 own tokens);
each core also receives a 256-token halo of the raw input before its own
range.  The attention window is 256, so the halo covers every key a core
needs; the SSM recurrence decay |A| < 0.1 makes state influence from before
the halo underflow fp32 entirely, so a zero-initialized scan warm-started
over the halo is exact.  No cross-core communication.

Layout: activations are feature-major [D, T] on the device (host transposes
in/out).  Weights and activations are bf16 (halves HBM traffic and doubles
DVE throughput; matmul cost on TRN2 is 1 row/cycle for bf16 and for fp32r
with free dim >= 256, so precision is the only trade — final rel err stays
~1e-3, well inside the 2e-2 gate).  LayerNorm statistics use an all-ones
stationary matmul (partition reduction, replicated over partitions), kept
in bf16 with f32 PSUM accumulation.  The SSM scan is one exact f32 DVE
tensor_tensor_scan.  V carries an interleaved ones column per head so each
PV matmul also emits the softmax denominator.  Elementwise epilogues are
spread across DVE / Pool(gpsimd) / Activation so no single engine gates
the Tensor engine; activation-table switches (sqrt/sigmoid/exp/gelu) are
prefetched with 1-element dummy ops during idle windows.
"""

import sys

try:
    import concourse.bass as bass  # noqa: F401
except Exception:
    sys.path.insert(0, "/opt/trn_rl_repo")

import numpy as np

import concourse.bass as bass  # noqa: F401
import concourse.mybir as mybir
from concourse import bacc, bass_utils
from concourse.tile import TileContext

F32 = mybir.dt.float32
F32R = mybir.dt.float32r
BF16 = mybir.dt.bfloat16
NPBF16 = mybir.dt.np(mybir.dt.bfloat16)

D = 1024
S = 128
H = 16
DH = 64
C = 256          # attention window / block size
T = 4096
NCORES = 8
TOWN = T // NCORES        # 512 own tokens per core
HALO = C                  # 256 halo tokens
TLOC = TOWN + HALO        # 768 local rows per core
EPS = 1e-5

# bias-pack column layout (f32 per-partition scalars)
BC_A = 0
BC_QKV = 1       # 24 cols
BC_GATE = 25     # 8
BC_DRIVE = 33    # 1
BC_O = 34        # 8
BC_B1 = 42       # 32
BC_B2 = 74       # 8
NBC = 82

T_TILES = [(0, 512), (512, 256)]
OWN0 = HALO


def _r(ap):
    return ap


def build_program(reps=1):
    nc = bacc.Bacc("TRN2", target_bir_lowering=False, debug=False)
    d_xT = nc.dram_tensor("xT", [D, TLOC], BF16, kind="ExternalInput").ap()
    d_mask0 = nc.dram_tensor("mask0", [128, 4 * C], BF16, kind="ExternalInput").ap()
    d_mask1 = nc.dram_tensor("mask1", [128, 4 * C], BF16, kind="ExternalInput").ap()
    d_bias = nc.dram_tensor("biaspack", [128, NBC], F32, kind="ExternalInput").ap()
    d_vbias = nc.dram_tensor("vbias", [1, D], F32R, kind="ExternalInput").ap()
    d_wqkv = nc.dram_tensor("wqkv", [D, 3 * D], BF16, kind="ExternalInput").ap()
    d_wgate = nc.dram_tensor("wgate", [D, D], BF16, kind="ExternalInput").ap()
    d_wdrive = nc.dram_tensor("wdrive", [D, S], BF16, kind="ExternalInput").ap()
    d_wo = nc.dram_tensor("wo", [D, D], BF16, kind="ExternalInput").ap()
    d_cw = nc.dram_tensor("cw", [S, D], F32R, kind="ExternalInput").ap()
    d_w1 = nc.dram_tensor("w1", [D, 4 * D], BF16, kind="ExternalInput").ap()
    d_w2 = nc.dram_tensor("w2", [4 * D, D], BF16, kind="ExternalInput").ap()
    d_out = nc.dram_tensor("outT", [D, TOWN], BF16, kind="ExternalOutput").ap()

    AF = mybir.ActivationFunctionType
    OP = mybir.AluOpType

    def persist(pool, shape, tag, dtype=BF16):
        return pool.tile(shape, dtype, tag=tag, name=tag, bufs=1)

    with TileContext(nc) as tc:
        for _rep in range(reps):
            with tc.tile_pool(name="const", bufs=1) as const, \
                 tc.tile_pool(name="xmid", bufs=1) as xm_pool:
                # const tiles: allocate now; DMAs for bias/masks/vbias are
                # issued AFTER the x-chunk DMAs (those gate the first matmul)
                biasp = persist(const, [128, NBC], "biasp", F32)
                mask0 = persist(const, [128, 4 * C], "mask0")
                mask1 = persist(const, [128, 4 * C], "mask1")
                vbrow = persist(const, [1, D], "vbrow", F32R)
                a_full = persist(const, [128, TLOC], "a_full", F32)
                eps_col = persist(const, [128, 1], "eps_col", F32)
                nc.vector.memset(eps_col[:], EPS)
                dummy = persist(const, [1, 1], "dummy", F32)

                def act_prefetch(func):
                    # 1-element activation to pull the act table in during an
                    # idle window instead of on the critical path
                    nc.scalar.activation(dummy[:], eps_col[0:1, :], func)

                # sqrt table first so the LN1 rstd sqrt needs no reload
                act_prefetch(AF.Sqrt)
                ones_ln = persist(const, [128, 128], "ones_ln", BF16)
                nc.vector.memset(ones_ln[:], 1.0 / D)
                onesr_f = persist(const, [1, 128], "onesr_f", F32)
                nc.vector.memset(onesr_f[:], 1.0)
                ones_row = persist(const, [1, 128], "ones_row", F32R)
                nc.scalar.activation(ones_row[:], onesr_f[:], AF.Copy,
                                     bias=0.0)

                def bias_col(idx):
                    return biasp[:, idx:idx + 1]

                def layernorm(xs, tfree, stats, spsum, out_pool, tagp,
                              sq_pool_cs=(), norm_pool_cs=(2, 6),
                              tiles=None):
                    """Stats via ones-matmul; normalize split DVE/Pool, bf16.

                    Normalization is emitted per (t-tile, chunk) so the first
                    t-tile's outputs exist before the second tile's stats are
                    done — downstream matmul chains start ~5us earlier.
                    """
                    mu = persist(stats, [128, tfree], f"mu{tagp}")
                    rstd = persist(stats, [128, tfree], f"rstd{tagp}")
                    var = persist(stats, [128, tfree], f"var{tagp}", F32)
                    outs = [persist(out_pool, [128, tfree], f"{tagp}{c}")
                            for c in range(8)]
                    ttl = tiles if tiles is not None else \
                        [(t0, tw) for (t0, tw) in T_TILES if t0 < tfree]
                    for t0, tw in ttl:
                        ps_mu = spsum.tile([128, 512], F32, tag="ln_mu")
                        ps_sq = spsum.tile([128, 512], F32, tag="ln_sq")
                        for c in range(8):
                            nc.tensor.matmul(ps_mu[:, :tw], _r(ones_ln[:]),
                                             _r(xs[c][:, t0:t0 + tw]),
                                             start=(c == 0), stop=(c == 7))
                        nc.vector.tensor_copy(mu[:, t0:t0 + tw], ps_mu[:, :tw])
                        for c in range(8):
                            sq = stats.tile([128, 512], BF16, tag="sq", bufs=3)
                            # square on DVE/Pool, not Act: keeps the Act
                            # queue clear of table-set churn
                            seng = (nc.gpsimd if (t0 > 0 or c in sq_pool_cs)
                                    else nc.vector)
                            seng.tensor_mul(sq[:, :tw], xs[c][:, t0:t0 + tw],
                                            xs[c][:, t0:t0 + tw])
                            nc.tensor.matmul(ps_sq[:, :tw], _r(ones_ln[:]),
                                             _r(sq[:, :tw]),
                                             start=(c == 0), stop=(c == 7))
                        nc.vector.tensor_mul(var[:, t0:t0 + tw],
                                             mu[:, t0:t0 + tw],
                                             mu[:, t0:t0 + tw])
                        nc.vector.tensor_sub(var[:, t0:t0 + tw],
                                             ps_sq[:, :tw], var[:, t0:t0 + tw])
                        nc.scalar.activation(var[:, t0:t0 + tw],
                                             var[:, t0:t0 + tw], AF.Sqrt,
                                             bias=eps_col[:])
                        with nc.allow_low_precision(
                                reason="bf16 rstd: 0.4% scale error on "
                                "normalized activations is benign"):
                            nc.vector.reciprocal(rstd[:, t0:t0 + tw],
                                                 var[:, t0:t0 + tw])
                        for c in range(8):
                            o = outs[c]
                            eng = nc.gpsimd if c in norm_pool_cs else nc.vector
                            eng.tensor_sub(o[:, t0:t0 + tw],
                                           xs[c][:, t0:t0 + tw],
                                           mu[:, t0:t0 + tw])
                            eng.tensor_mul(o[:, t0:t0 + tw],
                                           o[:, t0:t0 + tw],
                                           rstd[:, t0:t0 + tw])
                    return outs

                # ====== phase group A: LN1, projections, attention, fusion ====
                with tc.tile_pool(name="act1", bufs=1) as act1:
                    gs = [persist(act1, [128, TOWN], f"g{c}") for c in range(8)]
                    attns = [persist(act1, [128, TOWN], f"at{c}")
                             for c in range(8)]
                    driveT = persist(act1, [128, TLOC], "driveT", F32)
                    states = persist(act1, [128, TLOC], "states", F32R)
                    # x resident for the whole phase: LN1 input + residual.
                    # One wide tile, TWO DMAs total (HWDGE descriptor
                    # generation is a fixed 625ns per DMA instruction, so
                    # fewer/wider transfers beat per-chunk loads)
                    xpairs = [persist(act1, [128, 2 * TLOC], f"xp{i}")
                              for i in range(4)]
                    wc = persist(act1, [128, 1024], "wc", F32R)
                    wos = [persist(act1, [128, 4096], f"wo{mg}")
                           for mg in range(2)]
                    xts = [xpairs[c // 2][:, (c % 2) * TLOC:
                                          (c % 2 + 1) * TLOC]
                           for c in range(8)]
                    xos = [xpairs[c // 2][:, (c % 2) * TLOC + OWN0:
                                          (c % 2 + 1) * TLOC]
                           for c in range(8)]

                    with tc.tile_pool(name="act0", bufs=1) as act0:
                        kts = [persist(act0, [128, TLOC], f"k{c}") for c in range(8)]
                        vts = [persist(act0, [128, 16 * 65], f"v{c}")
                               for c in range(6)]
                        qts = [persist(act0, [128, TOWN], f"q{c}") for c in range(8)]
                        for tt in range(6):
                            vr = vts[tt][:].rearrange("p (h e) -> p h e", e=65)
                            nc.gpsimd.memset(vr[:, :, 64:65], 1.0)

                        with tc.tile_pool(name="xnp", bufs=1) as xnp, \
                             tc.tile_pool(name="lnst", bufs=1) as lnst:
                            for i in range(4):
                                nc.sync.dma_start(
                                    xpairs[i][:],
                                    d_xT[i * 256:(i + 1) * 256, :].rearrange(
                                        "(c p) t -> p c t", p=128))
                            # const DMAs after the x chunks they'd delay
                            # (masks wait further: not needed until attention)
                            nc.sync.dma_start(biasp[:], d_bias[:])
                            nc.sync.dma_start(vbrow[:], d_vbias[:])
                            with tc.tile_pool(name="lnpsum", bufs=2,
                                              space="PSUM") as lnp:
                                xns = layernorm(xts, TLOC, lnst, lnp, xnp, "xn")

                            # ---- projections -------------------------------
                            # one DMA per 8-chunk weight group: HWDGE costs a
                            # fixed 625ns per DMA instruction, so group loads
                            # through a strided (c p) m -> p (c m) pattern
                            with tc.tile_pool(name="wlin", bufs=8) as wp, \
                                 tc.tile_pool(name="linpsum", bufs=5,
                                              space="PSUM") as psum:

                                def load_wg(dram, m0, mw, tag="w", bufs=None):
                                    w = wp.tile([128, 8 * mw], BF16, tag=tag,
                                                bufs=bufs)
                                    nc.sync.dma_start(
                                        w[:],
                                        dram[0:1024, m0:m0 + mw].rearrange(
                                            "(c p) m -> p c m", p=128))
                                    return w

                                # All chains split by LN1 t-tile: every chain
                                # that only needs tile-0 xn (cols 0:512) is
                                # emitted before any chain touching tile 1,
                                # so the in-order PE queue streams behind the
                                # LN1 normalize instead of stalling on it.
                                wgates = [load_wg(d_wgate, mg * 512, 512)
                                          for mg in range(2)]
                                wks = [load_wg(d_wqkv, D + mg * 512, 512)
                                       for mg in range(2)]
                                wvs = [load_wg(d_wqkv, 2 * D + vc * 512, 512)
                                       for vc in range(2)]
                                wqs = [load_wg(d_wqkv, mg * 512, 512)
                                       for mg in range(2)]
                                wd = load_wg(d_wdrive, 0, 128, tag="wd",
                                             bufs=1)

                                def gate_q(t0, tw):
                                    # gate first: its sigmoids are the only
                                    # Act work here, so Act reaches the
                                    # exp-table prefetch early
                                    for wgrp, outs, bc0, act in (
                                            (wgates, gs, BC_GATE, True),
                                            (wqs, qts, BC_QKV, False)):
                                        for mg in range(2):
                                            for j in range(4):
                                                mc = mg * 4 + j
                                                ps = psum.tile([128, 512], F32,
                                                               tag="lin")
                                                for kc in range(8):
                                                    w0 = kc * 512 + j * 128
                                                    nc.tensor.matmul(
                                                        ps[:, :tw],
                                                        _r(wgrp[mg][:,
                                                           w0:w0 + 128]),
                                                        _r(xns[kc][:,
                                                           t0:t0 + tw]),
                                                        start=(kc == 0),
                                                        stop=(kc == 7))
                                                o0 = t0 - OWN0
                                                if act:
                                                    nc.scalar.activation(
                                                        outs[mc][:, o0:o0 + tw],
                                                        ps[:, :tw], AF.Sigmoid,
                                                        bias=bias_col(bc0 + mc))
                                                else:
                                                    nc.vector.tensor_scalar(
                                                        outs[mc][:, o0:o0 + tw],
                                                        ps[:, :tw],
                                                        bias_col(bc0 + mc),
                                                        None, OP.add)

                                def k_drive(t0, tw):
                                    for mg in range(2):
                                        for j in range(4):
                                            mc = mg * 4 + j
                                            ps = psum.tile([128, 512], F32,
                                                           tag="lin")
                                            for kc in range(8):
                                                w0 = kc * 512 + j * 128
                                                nc.tensor.matmul(
                                                    ps[:, :tw],
                                                    _r(wks[mg][:, w0:w0 + 128]),
                                                    _r(xns[kc][:, t0:t0 + tw]),
                                                    start=(kc == 0),
                                                    stop=(kc == 7))
                                            nc.vector.tensor_scalar(
                                                kts[mc][:, t0:t0 + tw],
                                                ps[:, :tw],
                                                bias_col(BC_QKV + 8 + mc),
                                                None, OP.add)
                                    ps = psum.tile([128, 512], F32, tag="lin")
                                    for kc in range(8):
                                        nc.tensor.matmul(
                                            ps[:, :tw],
                                            _r(wd[:, kc * 128:(kc + 1) * 128]),
                                            _r(xns[kc][:, t0:t0 + tw]),
                                            start=(kc == 0), stop=(kc == 7))
                                    nc.vector.tensor_scalar(
                                        driveT[:, t0:t0 + tw], ps[:, :tw],
                                        bias_col(BC_DRIVE), None, OP.add)

                                def v_block(tts):
                                    # V bias folded into the accumulation as
                                    # a rank-1 ones x vbrow matmul
                                    for vc in range(2):
                                        for tt in tts:
                                            t0 = tt * 128
                                            ps = psum.tile([128, 512], F32,
                                                           tag="lin")
                                            for kc in range(8):
                                                nc.tensor.matmul(
                                                    ps[:],
                                                    _r(xns[kc][:, t0:t0 + 128]),
                                                    _r(wvs[vc][:, kc * 512:
                                                               (kc + 1) * 512]),
                                                    start=(kc == 0), stop=False)
                                            nc.tensor.matmul(
                                                ps[:], ones_row[:, 0:128],
                                                vbrow[:,
                                                      vc * 512:(vc + 1) * 512],
                                                start=False, stop=True)
                                            vr = vts[tt][:].rearrange(
                                                "p (h e) -> p h e", e=65)
                                            nc.scalar.activation(
                                                vr[:, vc * 8:(vc + 1) * 8,
                                                   0:64],
                                                ps[:].rearrange(
                                                    "p (h e) -> p h e", e=64),
                                                AF.Copy, bias=0.0)

                                # tile-0-only chains
                                gate_q(OWN0, 512 - OWN0)
                                k_drive(0, 512)
                                v_block(range(4))
                                # tile-1 chains
                                gate_q(512, TLOC - 512)
                                act_prefetch(AF.Exp)
                                k_drive(512, TLOC - 512)
                                v_block((4, 5))
                        # xnp closed: xn freed before attention

                        # masks overlap late projections (x stays resident,
                        # no residual re-load needed)
                        nc.sync.dma_start(mask0[:], d_mask0[:])
                        nc.sync.dma_start(mask1[:], d_mask1[:])
                        # fusion + first MLP weights stream during attention
                        nc.sync.dma_start(wc[:], d_cw[:])
                        for mg in range(2):
                            nc.sync.dma_start(
                                wos[mg][:],
                                d_wo[0:1024,
                                     mg * 512:mg * 512 + 512].rearrange(
                                    "(c p) m -> p c m", p=128))
                        w1pre = persist(xm_pool, [128, 4096], "w1g0")
                        nc.sync.dma_start(
                            w1pre[:],
                            d_w1[0:1024, 0:512].rearrange(
                                "(c p) m -> p c m", p=128))
                        # scan coefficient, off the critical path on Pool
                        nc.gpsimd.memset(a_full[:], 1.0)
                        nc.gpsimd.tensor_scalar_mul(a_full[:], a_full[:],
                                                    biasp[:, BC_A:BC_A + 1])
                        # SSM scan emitted BEFORE attention: driveT is ready,
                        # and queueing it here keeps it off the back of the
                        # attention-heavy DVE queue, so states exist the
                        # moment the WO/Cw matmuls want them
                        nc.vector.tensor_tensor_scan(states[:], a_full[:],
                                                     driveT[:], 0.0,
                                                     OP.mult, OP.add)

                        # ---- windowed attention ----------------------------
                        # processed per head PAIR (both heads of one cch):
                        # one [2,128] selector matmul broadcasts both heads'
                        # softmax reciprocals at once, one Act copy stages
                        # them in SBUF (HW: DVE may read only one PSUM
                        # operand, Pool none)
                        with tc.tile_pool(name="apsum", bufs=2,
                                          space="PSUM") as apsum, \
                             tc.tile_pool(name="posum", bufs=3,
                                          space="PSUM") as posum, \
                             tc.tile_pool(name="rbsum", bufs=1,
                                          space="PSUM") as rbsum, \
                             tc.tile_pool(name="ptp", bufs=3) as ptp, \
                             tc.tile_pool(name="rp", bufs=4) as rp:
                            # 0/1 selector rows: rb2 = selA^T@rrA + selB^T@rrB
                            # broadcasts both heads' reciprocals in one
                            # [128,C] accumulation with a partition-0 dst
                            selA = persist(rp, [1, 128], "selA")
                            nc.gpsimd.memset(selA[:], 0.0)
                            nc.gpsimd.memset(selA[0:1, 0:64], 1.0)
                            selB = persist(rp, [1, 128], "selB")
                            nc.gpsimd.memset(selB[:], 0.0)
                            nc.gpsimd.memset(selB[0:1, 64:128], 1.0)
                            for b in range(2):
                                mask = mask0 if b == 0 else mask1
                                for cch in range(8):
                                    pos = []
                                    rrs = []
                                    for hi in range(2):
                                        h = 2 * cch + hi
                                        half = hi * 64
                                        st4 = apsum.tile([128, 4 * C], F32,
                                                         tag="st4")
                                        for kc in range(4):
                                            k0 = C * b + 128 * kc
                                            nc.tensor.matmul(
                                                st4[:, kc * C:(kc + 1) * C],
                                                _r(kts[cch][half:half + 64,
                                                            k0:k0 + 128]),
                                                _r(qts[cch][half:half + 64,
                                                            C * b:C * (b + 1)]),
                                                start=True, stop=True)
                                        pt4 = ptp.tile([128, 4 * C], BF16,
                                                       tag="pt4")
                                        nc.scalar.activation(
                                            pt4[:], st4[:], AF.Exp,
                                            scale=float(1.0 / np.sqrt(DH)))
                                        meng = nc.vector if hi == 0 \
                                            else nc.gpsimd
                                        meng.tensor_mul(pt4[:], pt4[:],
                                                        mask[:])
                                        po = posum.tile([65, C], F32,
                                                        tag="po")
                                        for kc in range(4):
                                            nc.tensor.matmul(
                                                po[:],
                                                _r(vts[2 * b + kc][:,
                                                   h * 65:(h + 1) * 65]),
                                                _r(pt4[:,
                                                   kc * C:(kc + 1) * C]),
                                                start=(kc == 0),
                                                stop=(kc == 3))
                                        pos.append(po)
                                        rr = rp.tile([1, C], BF16, tag="rr")
                                        with nc.allow_low_precision(
                                                reason="bf16 rounding of "
                                                "softmax denominators is "
                                                "benign"):
                                            nc.vector.reciprocal(
                                                rr[:], po[64:65, :])
                                        rrs.append(rr)
                                    rb2 = rbsum.tile([128, C], F32, tag="rb")
                                    for hi, sel in ((0, selA), (1, selB)):
                                        nc.tensor.matmul(
                                            rb2[:], sel[:], rrs[hi][:],
                                            start=(hi == 0), stop=(hi == 1))
                                    rs2 = rp.tile([128, C], BF16, tag="rs")
                                    nc.scalar.activation(rs2[:], rb2[:],
                                                         AF.Copy, bias=0.0)
                                    for hi in range(2):
                                        half = hi * 64
                                        nc.vector.tensor_mul(
                                            attns[cch][half:half + 64,
                                                       C * b:C * (b + 1)],
                                            pos[hi][0:64, :],
                                            rs2[half:half + 64, :])
                    # act0 closed: k/v/q freed

                    # ---- SSM output + WO + fusion --------------------------
                    act_prefetch(AF.Sqrt)  # LN2 table, during WO window
                    with tc.tile_pool(name="fus", bufs=1) as fus, \
                         tc.tile_pool(name="wfus", bufs=10) as wf, \
                         tc.tile_pool(name="spsum", bufs=4, space="PSUM") as sp:
                        ys = [persist(fus, [128, TOWN], f"y{c}") for c in range(8)]
                        zs = [persist(fus, [128, TOWN], f"z{c}") for c in range(8)]
                        for mc in range(8):
                            ps = sp.tile([128, 512], F32, tag="s")
                            nc.tensor.matmul(
                                ps[:],
                                _r(wc[:, mc * 128:(mc + 1) * 128]),
                                _r(states[:, OWN0:OWN0 + TOWN]),
                                start=True, stop=True)
                            nc.scalar.activation(ys[mc][:], ps[:], AF.Copy,
                                                 bias=0.0)
                            eng = nc.gpsimd if mc % 2 else nc.vector
                            eng.tensor_add(zs[mc][:], ys[mc][:], xos[mc])
                        xms = [persist(xm_pool, [128, TOWN], f"xm{c}")
                               for c in range(8)]
                        half1 = []
                        for mg in range(2):
                            for j in range(4):
                                mc = mg * 4 + j
                                ps = sp.tile([128, 512], F32, tag="s")
                                for kc in range(8):
                                    w0 = kc * 512 + j * 128
                                    nc.tensor.matmul(
                                        ps[:],
                                        _r(wos[mg][:, w0:w0 + 128]),
                                        _r(attns[kc][:]),
                                        start=(kc == 0), stop=(kc == 7))
                                # epilogue split by token halves: half-0 of
                                # every chunk lands first so LN2 stats can
                                # start while half-1 is still in flight
                                xm = xms[mc]

                                def epi(mc, ps, h0, hw):
                                    xm = xms[mc]
                                    nc.vector.scalar_tensor_tensor(
                                        xm[:, h0:h0 + hw], ps[:, h0:h0 + hw],
                                        bias_col(BC_O + mc),
                                        ys[mc][:, h0:h0 + hw],
                                        op0=OP.add, op1=OP.subtract)
                                    eng = (nc.gpsimd if mc in (2, 5, 7)
                                           else nc.vector)
                                    eng.tensor_mul(xm[:, h0:h0 + hw],
                                                   xm[:, h0:h0 + hw],
                                                   gs[mc][:, h0:h0 + hw])
                                    eng.tensor_add(xm[:, h0:h0 + hw],
                                                   xm[:, h0:h0 + hw],
                                                   zs[mc][:, h0:h0 + hw])

                                epi(mc, ps, 0, 256)
                                half1.append((mc, ps))
                        for mc, ps in half1:
                            epi(mc, ps, 256, 256)
                # act1 closed: g/attn/drive/states/xos freed

                # ====== phase group B: LN2 + MLP ==============================
                with tc.tile_pool(name="xn2p", bufs=1) as xn2p, \
                     tc.tile_pool(name="wmlp", bufs=3) as wm, \
                     tc.tile_pool(name="w2p", bufs=1) as w2p:

                    def load_w1g(mg):
                        w = wm.tile([128, 4096], BF16, tag="w1")
                        nc.sync.dma_start(
                            w[:],
                            d_w1[0:1024, mg * 512:mg * 512 + 512].rearrange(
                                "(c p) m -> p c m", p=128))
                        return w

                    with tc.tile_pool(name="lnst2", bufs=1) as lnst2:
                        with tc.tile_pool(name="ln2psum", bufs=2,
                                          space="PSUM") as lnp2:
                            xn2s = layernorm(xms, TOWN, lnst2, lnp2, xn2p, "h",
                                             sq_pool_cs=(1, 4, 6),
                                             norm_pool_cs=(2, 5),
                                             tiles=[(0, 256), (256, 256)])
                        act_prefetch(AF.Gelu)
                    with tc.tile_pool(name="hTp", bufs=1) as hTp:
                        hts = [persist(hTp, [128, TOWN], f"ht{c}")
                               for c in range(32)]
                        with tc.tile_pool(name="m1psum", bufs=6,
                                          space="PSUM") as mp1:
                            for mg in range(8):
                                ws = w1pre if mg == 0 else load_w1g(mg)
                                pss = [mp1.tile([128, 512], F32, tag="m",
                                                name=f"m{j}")
                                       for j in range(4)]
                                for h0 in (0, 256):
                                    for j in range(4):
                                        for kc in range(8):
                                            w0 = kc * 512 + j * 128
                                            nc.tensor.matmul(
                                                pss[j][:, h0:h0 + 256],
                                                _r(ws[:, w0:w0 + 128]),
                                                _r(xn2s[kc][:, h0:h0 + 256]),
                                                start=(kc == 0),
                                                stop=(kc == 7))
                                for j in range(4):
                                    mc = mg * 4 + j
                                    nc.scalar.activation(
                                        hts[mc][:], pss[j][:], AF.Gelu,
                                        bias=bias_col(BC_B1 + mc))
                        # all of W2 resident (64KB/partition, bf16), ONE DMA
                        # issued while MLP1 computes; mc-outer chains below
                        # let each output's epilogue+store overlap the next
                        w2all = persist(w2p, [128, 32 * 1024], "w2all")
                        nc.sync.dma_start(
                            w2all[:],
                            d_w2[0:4096, 0:1024].rearrange(
                                "(c p) m -> p c m", p=128))
                        with tc.tile_pool(name="m2psum", bufs=3,
                                          space="PSUM") as mp2, \
                             tc.tile_pool(name="outp", bufs=3) as outp:
                            for mc in range(8):
                                ps = mp2.tile([128, 512], F32, tag="o")
                                for kc in range(32):
                                    w0 = kc * 1024 + mc * 128
                                    nc.tensor.matmul(
                                        ps[:],
                                        _r(w2all[:, w0:w0 + 128]),
                                        _r(hts[kc][:]),
                                        start=(kc == 0), stop=(kc == 31))
                                oc = outp.tile([128, TOWN], BF16, tag="oc")
                                nc.vector.scalar_tensor_tensor(
                                    oc[:], ps[:], bias_col(BC_B2 + mc),
                                    xms[mc][:], op0=OP.add, op1=OP.add)
                                nc.sync.dma_start(
                                    d_out[mc * 128:(mc + 1) * 128, :], oc[:])

    nc.compile()
    return nc


def _make_masks():
    qi = np.arange(C)[:, None]
    kk = np.arange(2 * C)[None, :]
    band = (kk > qi) & (kk <= qi + C)
    first = band & (kk >= C)

    def pack(m):                       # [C, 2C] -> [128, 4*C] k-chunk-major
        mt = m.T.astype(NPBF16)        # [2C, C]
        return np.ascontiguousarray(
            mt.reshape(4, 128, C).transpose(1, 0, 2).reshape(128, 4 * C))

    return pack(first), pack(band)


def _prep_inputs(x, ln1_g, ln1_b, ln2_g, ln2_b, W_qkv, W_O, b_O, W_ug, b_ug,
                 B_w, A, C_w, mlp_W1, mlp_b1, mlp_W2, mlp_b2):
    f = np.float32
    g1 = np.asarray(ln1_g, f)
    b1 = np.asarray(ln1_b, f)
    W_qkv = np.asarray(W_qkv, f)
    W_qkv_e = g1[:, None] * W_qkv
    b_qkv_e = b1 @ W_qkv
    W_ug = np.asarray(W_ug, f)
    B_w = np.asarray(B_w, f)
    b_ug = np.asarray(b_ug, f)
    W_drive_raw = B_w + W_ug[:, :S]
    W_drive_e = g1[:, None] * W_drive_raw
    b_drive_e = b1 @ W_drive_raw + b_ug[:S]
    W_gate_e = g1[:, None] * W_ug[:, S:]
    b_gate_e = b1 @ W_ug[:, S:] + b_ug[S:]
    g2 = np.asarray(ln2_g, f)
    b2l = np.asarray(ln2_b, f)
    mlp_W1 = np.asarray(mlp_W1, f)
    W1_e = g2[:, None] * mlp_W1
    b1_e = b2l @ mlp_W1 + np.asarray(mlp_b1, f)

    biaspack = np.zeros((128, NBC), f)
    biaspack[:, BC_A] = np.asarray(A, f)
    biaspack[:, BC_QKV:BC_QKV + 24] = b_qkv_e.reshape(24, 128).T
    biaspack[:, BC_GATE:BC_GATE + 8] = b_gate_e.reshape(8, 128).T
    biaspack[:, BC_DRIVE] = b_drive_e
    biaspack[:, BC_O:BC_O + 8] = np.asarray(b_O, f).reshape(8, 128).T
    biaspack[:, BC_B1:BC_B1 + 32] = b1_e.reshape(32, 128).T
    biaspack[:, BC_B2:BC_B2 + 8] = np.asarray(mlp_b2, f).reshape(8, 128).T
    vbias = np.ascontiguousarray(b_qkv_e[2 * D:].reshape(1, D))

    m_first, m_band = _make_masks()
    xTfull = np.ascontiguousarray(np.asarray(x, f)[0].T.astype(NPBF16))

    def bf(a):
        return np.ascontiguousarray(np.asarray(a, f).astype(NPBF16))

    shared = {
        "biaspack": biaspack, "vbias": vbias,
        "wqkv": bf(W_qkv_e),
        "wgate": bf(W_gate_e),
        "wdrive": bf(W_drive_e),
        "wo": bf(W_O),
        "cw": np.ascontiguousarray(np.asarray(C_w, f)),
        "w1": bf(W1_e),
        "w2": bf(mlp_W2),
        "mask1": m_band,
    }
    in_maps = []
    for i in range(NCORES):
        t0 = i * TOWN
        xT = np.zeros((D, TLOC), NPBF16)
        lo = max(0, t0 - HALO)
        xT[:, HALO - (t0 - lo):HALO] = xTfull[:, lo:t0]
        xT[:, HALO:] = xTfull[:, t0:t0 + TOWN]
        m0 = m_first if i == 0 else m_band
        in_maps.append({**shared, "xT": np.ascontiguousarray(xT), "mask0": m0})
    return in_maps


_CACHED_NC = None


def get_nc():
    global _CACHED_NC
    if _CACHED_NC is None:
        _CACHED_NC = build_program()
    return _CACHED_NC


def kernel(**inputs):
    nc = get_nc()
    in_maps = _prep_inputs(**inputs)
    res = bass_utils.run_bass_kernel_spmd(nc, in_maps,
                                          core_ids=list(range(NCORES)))
    out = np.empty((1, T, D), np.float32)
    for i in range(NCORES):
        out[0, i * TOWN:(i + 1) * TOWN, :] = \
            res.results[i]["outT"].astype(np.float32).T
    return out


# revision 53
# speedup vs baseline: 1.4610x; 1.0382x over previous
"""DPA+SSM block kernel for 8 Trainium2 NeuronCores.

Sharding: data-parallel over the sequence (T=4096 -> 8 x 512 own tokens);
each core also receives a 256-token halo of the raw input before its own
range.  The attention window is 256, so the halo covers every key a core
needs; the SSM recurrence decay |A| < 0.1 makes state influence from before
the halo underflow fp32 entirely, so a zero-initialized scan warm-started
over the halo is exact.  No cross-core communication.

Layout: activations are feature-major [D, T] on the device (host transposes
in/out).  Weights and activations are bf16 (halves HBM traffic and doubles
DVE throughput; matmul cost on TRN2 is 1 row/cycle for bf16 and for fp32r
with free dim >= 256, so precision is the only trade — final rel err stays
~1e-3, well inside the 2e-2 gate).  LayerNorm statistics use an all-ones
stationary matmul (partition reduction, replicated over partitions), kept
in bf16 with f32 PSUM accumulation.  The SSM scan is one exact f32 DVE
tensor_tensor_scan.  V carries an interleaved ones column per head so each
PV matmul also emits the softmax denominator.  Elementwise epilogues are
spread across DVE / Pool(gpsimd) / Activation so no single engine gates
the Tensor engine; activation-table switches (sqrt/sigmoid/exp/gelu) are
prefetched with 1-element dummy ops during idle windows.
"""

import sys

try:
    import concourse.bass as bass  # noqa: F401
except Exception:
    sys.path.insert(0, "/opt/trn_rl_repo")

import numpy as np

import concourse.bass as bass  # noqa: F401
import concourse.mybir as mybir
from concourse import bacc, bass_utils
from concourse.tile import TileContext

F32 = mybir.dt.float32
F32R = mybir.dt.float32r
BF16 = mybir.dt.bfloat16
NPBF16 = mybir.dt.np(mybir.dt.bfloat16)

D = 1024
S = 128
H = 16
DH = 64
C = 256          # attention window / block size
T = 4096
NCORES = 8
TOWN = T // NCORES        # 512 own tokens per core
HALO = C                  # 256 halo tokens
TLOC = TOWN + HALO        # 768 local rows per core
EPS = 1e-5

# bias-pack column layout (f32 per-partition scalars)
BC_A = 0
BC_QKV = 1       # 24 cols
BC_GATE = 25     # 8
BC_DRIVE = 33    # 1
BC_O = 34        # 8
BC_B1 = 42       # 32
BC_B2 = 74       # 8
NBC = 82

T_TILES = [(0, 512), (512, 256)]
OWN0 = HALO


def _r(ap):
    return ap


def build_program(reps=1):
    nc = bacc.Bacc("TRN2", target_bir_lowering=False, debug=False)
    d_xT = nc.dram_tensor("xT", [D, TLOC], BF16, kind="ExternalInput").ap()
    d_mask0 = nc.dram_tensor("mask0", [128, 4 * C], BF16, kind="ExternalInput").ap()
    d_mask1 = nc.dram_tensor("mask1", [128, 4 * C], BF16, kind="ExternalInput").ap()
    d_bias = nc.dram_tensor("biaspack", [128, NBC], F32, kind="ExternalInput").ap()
    d_vbias = nc.dram_tensor("vbias", [1, D], F32R, kind="ExternalInput").ap()
    d_wqkv = nc.dram_tensor("wqkv", [D, 3 * D], BF16, kind="ExternalInput").ap()
    d_wgate = nc.dram_tensor("wgate", [D, D], BF16, kind="ExternalInput").ap()
    d_wdrive = nc.dram_tensor("wdrive", [D, S], BF16, kind="ExternalInput").ap()
    d_wo = nc.dram_tensor("wo", [D, D], BF16, kind="ExternalInput").ap()
    d_cw = nc.dram_tensor("cw", [S, D], F32R, kind="ExternalInput").ap()
    d_w1 = nc.dram_tensor("w1", [D, 4 * D], BF16, kind="ExternalInput").ap()
    d_w2 = nc.dram_tensor("w2", [4 * D, D], BF16, kind="ExternalInput").ap()
    d_out = nc.dram_tensor("outT", [D, TOWN], BF16, kind="ExternalOutput").ap()

    AF = mybir.ActivationFunctionType
    OP = mybir.AluOpType

    def persist(pool, shape, tag, dtype=BF16):
        return pool.tile(shape, dtype, tag=tag, name=tag, bufs=1)

    with TileContext(nc) as tc:
        for _rep in range(reps):
            with tc.tile_pool(name="const", bufs=1) as const, \
                 tc.tile_pool(name="xmid", bufs=1) as xm_pool:
                # const tiles: allocate now; DMAs for bias/masks/vbias are
                # issued AFTER the x-chunk DMAs (those gate the first matmul)
                biasp = persist(const, [128, NBC], "biasp", F32)
                mask0 = persist(const, [128, 4 * C], "mask0")
                mask1 = persist(const, [128, 4 * C], "mask1")
                vbrow = persist(const, [1, D], "vbrow", F32R)
                a_full = persist(const, [128, TLOC], "a_full", F32)
                eps_col = persist(const, [128, 1], "eps_col", F32)
                nc.vector.memset(eps_col[:], EPS)
                dummy = persist(const, [1, 1], "dummy", F32)

                def act_prefetch(func):
                    # 1-element activation to pull the act table in during an
                    # idle window instead of on the critical path
                    nc.scalar.activation(dummy[:], eps_col[0:1, :], func)

                # sqrt table first so the LN1 rstd sqrt needs no reload
                act_prefetch(AF.Sqrt)
                ones_ln = persist(const, [128, 128], "ones_ln", BF16)
                nc.vector.memset(ones_ln[:], 1.0 / D)
                onesr_f = persist(const, [1, 128], "onesr_f", F32)
                nc.vector.memset(onesr_f[:], 1.0)
                ones_row = persist(const, [1, 128], "ones_row", F32R)
                nc.scalar.activation(ones_row[:], onesr_f[:], AF.Copy,
                                     bias=0.0)

                def bias_col(idx):
                    return biasp[:, idx:idx + 1]

                def layernorm(xs, tfree, stats, spsum, out_pool, tagp,
                              sq_pool_cs=(), norm_pool_cs=(2, 6),
                              tiles=None):
                    """Stats via ones-matmul; normalize split DVE/Pool, bf16.

                    Normalization is emitted per (t-tile, chunk) so the first
                    t-tile's outputs exist before the second tile's stats are
                    done — downstream matmul chains start ~5us earlier.
                    """
                    mu = persist(stats, [128, tfree], f"mu{tagp}")
                    rstd = persist(stats, [128, tfree], f"rstd{tagp}")
                    var = persist(stats, [128, tfree], f"var{tagp}", F32)
                    outs = [persist(out_pool, [128, tfree], f"{tagp}{c}")
                            for c in range(8)]
                    ttl = tiles if tiles is not None else \
                        [(t0, tw) for (t0, tw) in T_TILES if t0 < tfree]
                    for t0, tw in ttl:
                        ps_mu = spsum.tile([128, 512], F32, tag="ln_mu")
                        ps_sq = spsum.tile([128, 512], F32, tag="ln_sq")
                        for c in range(8):
                            nc.tensor.matmul(ps_mu[:, :tw], _r(ones_ln[:]),
                                             _r(xs[c][:, t0:t0 + tw]),
                                             start=(c == 0), stop=(c == 7))
                        nc.vector.tensor_copy(mu[:, t0:t0 + tw], ps_mu[:, :tw])
                        for c in range(8):
                            sq = stats.tile([128, 512], BF16, tag="sq", bufs=3)
                            # square on DVE/Pool, not Act: keeps the Act
                            # queue clear of table-set churn
                            seng = (nc.gpsimd if c in sq_pool_cs
                                    else nc.vector)
                            seng.tensor_mul(sq[:, :tw], xs[c][:, t0:t0 + tw],
                                            xs[c][:, t0:t0 + tw])
                            nc.tensor.matmul(ps_sq[:, :tw], _r(ones_ln[:]),
                                             _r(sq[:, :tw]),
                                             start=(c == 0), stop=(c == 7))
                        nc.vector.tensor_mul(var[:, t0:t0 + tw],
                                             mu[:, t0:t0 + tw],
                                             mu[:, t0:t0 + tw])
                        nc.vector.tensor_sub(var[:, t0:t0 + tw],
                                             ps_sq[:, :tw], var[:, t0:t0 + tw])
                        nc.scalar.activation(var[:, t0:t0 + tw],
                                             var[:, t0:t0 + tw], AF.Sqrt,
                                             bias=eps_col[:])
                        with nc.allow_low_precision(
                                reason="bf16 rstd: 0.4% scale error on "
                                "normalized activations is benign"):
                            nc.vector.reciprocal(rstd[:, t0:t0 + tw],
                                                 var[:, t0:t0 + tw])
                        for c in range(8):
                            o = outs[c]
                            eng = nc.gpsimd if c in norm_pool_cs else nc.vector
                            eng.tensor_sub(o[:, t0:t0 + tw],
                                           xs[c][:, t0:t0 + tw],
                                           mu[:, t0:t0 + tw])
                            eng.tensor_mul(o[:, t0:t0 + tw],
                                           o[:, t0:t0 + tw],
                                           rstd[:, t0:t0 + tw])
                    return outs

                # ====== phase group A: LN1, projections, attention, fusion ====
                with tc.tile_pool(name="act1", bufs=1) as act1:
                    gs = [persist(act1, [128, TOWN], f"g{c}") for c in range(8)]
                    attns = [persist(act1, [128, TOWN], f"at{c}")
                             for c in range(8)]
                    driveT = persist(act1, [128, TLOC], "driveT", F32)
                    states = persist(act1, [128, TLOC], "states", F32R)
                    # x resident for the whole phase: LN1 input + residual.
                    # One wide tile, TWO DMAs total (HWDGE descriptor
                    # generation is a fixed 625ns per DMA instruction, so
                    # fewer/wider transfers beat per-chunk loads)
                    wc = persist(act1, [128, 1024], "wc", F32R)
                    wos = [persist(act1, [128, 4096], f"wo{mg}")
                           for mg in range(2)]
                    xpairs = [persist(act1, [128, 2 * TLOC], f"xp{i}")
                              for i in range(4)]
                    xts = [xpairs[c // 2][:, (c % 2) * TLOC:
                                          (c % 2 + 1) * TLOC]
                           for c in range(8)]
                    xos = [xpairs[c // 2][:, (c % 2) * TLOC + OWN0:
                                          (c % 2 + 1) * TLOC]
                           for c in range(8)]

                    with tc.tile_pool(name="act0", bufs=1) as act0:
                        kt0 = [persist(act0, [128, 512], f"k0_{c}")
                               for c in range(8)]
                        kt1 = [persist(act0, [128, 256], f"k1_{c}")
                               for c in range(8)]
                        vts = [persist(act0, [128, 16 * 65], f"v{c}")
                               for c in range(6)]
                        qts = [[persist(act0, [128, 256], f"q{hb}_{c}")
                                for c in range(8)] for hb in range(2)]
                        for tt in range(6):
                            vr = vts[tt][:].rearrange("p (h e) -> p h e", e=65)
                            nc.gpsimd.memset(vr[:, :, 64:65], 1.0)

                        with tc.tile_pool(name="xnp", bufs=1) as xnp, \
                             tc.tile_pool(name="lnst", bufs=1) as lnst:
                            for i in range(4):
                                nc.sync.dma_start(
                                    xpairs[i][:],
                                    d_xT[i * 256:(i + 1) * 256, :].rearrange(
                                        "(c p) t -> p c t", p=128))
                            with tc.tile_pool(name="lnpsum", bufs=2,
                                              space="PSUM") as lnp:
                                xns = layernorm(xts, TLOC, lnst, lnp, xnp, "xn")

                            # ---- projections -------------------------------
                            # one DMA per 8-chunk weight group: HWDGE costs a
                            # fixed 625ns per DMA instruction, so group loads
                            # through a strided (c p) m -> p (c m) pattern
                            with tc.tile_pool(name="wlin", bufs=8) as wp, \
                                 tc.tile_pool(name="linpsum", bufs=5,
                                              space="PSUM") as psum:

                                def load_wg(dram, m0, mw, tag="w", bufs=None):
                                    w = wp.tile([128, 8 * mw], BF16, tag=tag,
                                                bufs=bufs)
                                    nc.sync.dma_start(
                                        w[:],
                                        dram[0:1024, m0:m0 + mw].rearrange(
                                            "(c p) m -> p c m", p=128))
                                    return w

                                # All chains split by LN1 t-tile: every chain
                                # that only needs tile-0 xn (cols 0:512) is
                                # emitted before any chain touching tile 1,
                                # so the in-order PE queue streams behind the
                                # LN1 normalize instead of stalling on it.
                                # load order matches PE consumption order
                                # in the t0 phase: gate -> Q -> K -> V
                                wgates = [load_wg(d_wgate, mg * 512, 512)
                                          for mg in range(2)]
                                nc.sync.dma_start(biasp[:], d_bias[:])
                                wqs = [load_wg(d_wqkv, mg * 512, 512)
                                       for mg in range(2)]
                                wks = [load_wg(d_wqkv, D + mg * 512, 512)
                                       for mg in range(2)]
                                nc.sync.dma_start(vbrow[:], d_vbias[:])
                                wvs = [load_wg(d_wqkv, 2 * D + vc * 512, 512)
                                       for vc in range(2)]
                                wd = load_wg(d_wdrive, 0, 128, tag="wd",
                                             bufs=1)

                                def gate_q(t0, tw):
                                    # gate first: its sigmoids are the only
                                    # Act work here, so Act reaches the
                                    # exp-table prefetch early
                                    for wgrp, outs, bc0, act in (
                                            (wgates, gs, BC_GATE, True),
                                            (wqs, qts, BC_QKV, False)):
                                        for mg in range(2):
                                            for j in range(4):
                                                mc = mg * 4 + j
                                                ps = psum.tile([128, 512], F32,
                                                               tag="lin")
                                                for kc in range(8):
                                                    w0 = kc * 512 + j * 128
                                                    nc.tensor.matmul(
                                                        ps[:, :tw],
                                                        _r(wgrp[mg][:,
                                                           w0:w0 + 128]),
                                                        _r(xns[kc][:,
                                                           t0:t0 + tw]),
                                                        start=(kc == 0),
                                                        stop=(kc == 7))
                                                o0 = t0 - OWN0
                                                if act:
                                                    nc.scalar.activation(
                                                        outs[mc][:, o0:o0 + tw],
                                                        ps[:, :tw], AF.Sigmoid,
                                                        bias=bias_col(bc0 + mc))
                                                else:
                                                    qdst = outs[0 if o0 == 0
                                                                else 1][mc]
                                                    nc.vector.tensor_scalar(
                                                        qdst[:, :tw],
                                                        ps[:, :tw],
                                                        bias_col(bc0 + mc),
                                                        None, OP.add)

                                def k_drive(t0, tw):
                                    for mg in range(2):
                                        for j in range(4):
                                            mc = mg * 4 + j
                                            ps = psum.tile([128, 512], F32,
                                                           tag="lin")
                                            for kc in range(8):
                                                w0 = kc * 512 + j * 128
                                                nc.tensor.matmul(
                                                    ps[:, :tw],
                                                    _r(wks[mg][:, w0:w0 + 128]),
                                                    _r(xns[kc][:, t0:t0 + tw]),
                                                    start=(kc == 0),
                                                    stop=(kc == 7))
                                            kdst = (kt0[mc][:, t0:t0 + tw]
                                                    if t0 == 0 else
                                                    kt1[mc][:, :tw])
                                            nc.vector.tensor_scalar(
                                                kdst, ps[:, :tw],
                                                bias_col(BC_QKV + 8 + mc),
                                                None, OP.add)
                                    ps = psum.tile([128, 512], F32, tag="lin")
                                    for kc in range(8):
                                        nc.tensor.matmul(
                                            ps[:, :tw],
                                            _r(wd[:, kc * 128:(kc + 1) * 128]),
                                            _r(xns[kc][:, t0:t0 + tw]),
                                            start=(kc == 0), stop=(kc == 7))
                                    nc.vector.tensor_scalar(
                                        driveT[:, t0:t0 + tw], ps[:, :tw],
                                        bias_col(BC_DRIVE), None, OP.add)

                                def v_block(tts):
                                    # V bias folded into the accumulation as
                                    # a rank-1 ones x vbrow matmul
                                    for vc in range(2):
                                        for tt in tts:
                                            t0 = tt * 128
                                            ps = psum.tile([128, 512], F32,
                                                           tag="lin")
                                            for kc in range(8):
                                                nc.tensor.matmul(
                                                    ps[:],
                                                    _r(xns[kc][:, t0:t0 + 128]),
                                                    _r(wvs[vc][:, kc * 512:
                                                               (kc + 1) * 512]),
                                                    start=(kc == 0), stop=False)
                                            nc.tensor.matmul(
                                                ps[:], ones_row[:, 0:128],
                                                vbrow[:,
                                                      vc * 512:(vc + 1) * 512],
                                                start=False, stop=True)
                                            vr = vts[tt][:].rearrange(
                                                "p (h e) -> p h e", e=65)
                                            nc.scalar.activation(
                                                vr[:, vc * 8:(vc + 1) * 8,
                                                   0:64],
                                                ps[:].rearrange(
                                                    "p (h e) -> p h e", e=64),
                                                AF.Copy, bias=0.0)

                                # tile-0-only chains
                                gate_q(OWN0, 512 - OWN0)
                                k_drive(0, 512)
                                v_block(range(4))
                                # tile-1 chains
                                gate_q(512, TLOC - 512)
                                act_prefetch(AF.Exp)
                                k_drive(512, TLOC - 512)
                                v_block((4, 5))
                        # xnp closed: xn freed before attention

                        # masks overlap late projections (x stays resident,
                        # no residual re-load needed)
                        nc.sync.dma_start(mask0[:], d_mask0[:])
                        nc.sync.dma_start(mask1[:], d_mask1[:])
                        # fusion + first MLP weights stream during attention
                        nc.sync.dma_start(wc[:], d_cw[:])
                        for mg in range(2):
                            nc.sync.dma_start(
                                wos[mg][:],
                                d_wo[0:1024,
                                     mg * 512:mg * 512 + 512].rearrange(
                                    "(c p) m -> p c m", p=128))
                        w1pre = persist(xm_pool, [128, 4096], "w1g0")
                        nc.sync.dma_start(
                            w1pre[:],
                            d_w1[0:1024, 0:512].rearrange(
                                "(c p) m -> p c m", p=128))
                        # scan coefficient, off the critical path on Pool
                        nc.gpsimd.memset(a_full[:], 1.0)
                        nc.gpsimd.tensor_scalar_mul(a_full[:], a_full[:],
                                                    biasp[:, BC_A:BC_A + 1])
                        # SSM scan emitted BEFORE attention: driveT is ready,
                        # and queueing it here keeps it off the back of the
                        # attention-heavy DVE queue, so states exist the
                        # moment the WO/Cw matmuls want them
                        nc.vector.tensor_tensor_scan(states[:], a_full[:],
                                                     driveT[:], 0.0,
                                                     OP.mult, OP.add)

                        # ---- windowed attention ----------------------------
                        # processed per head PAIR (both heads of one cch):
                        # one [2,128] selector matmul broadcasts both heads'
                        # softmax reciprocals at once, one Act copy stages
                        # them in SBUF (HW: DVE may read only one PSUM
                        # operand, Pool none)
                        with tc.tile_pool(name="apsum", bufs=2,
                                          space="PSUM") as apsum, \
                             tc.tile_pool(name="posum", bufs=3,
                                          space="PSUM") as posum, \
                             tc.tile_pool(name="rbsum", bufs=1,
                                          space="PSUM") as rbsum, \
                             tc.tile_pool(name="ptp", bufs=3) as ptp, \
                             tc.tile_pool(name="rp", bufs=4) as rp:
                            # 0/1 selector rows: rb2 = selA^T@rrA + selB^T@rrB
                            # broadcasts both heads' reciprocals in one
                            # [128,C] accumulation with a partition-0 dst
                            selA = persist(rp, [1, 128], "selA")
                            nc.gpsimd.memset(selA[:], 0.0)
                            nc.gpsimd.memset(selA[0:1, 0:64], 1.0)
                            selB = persist(rp, [1, 128], "selB")
                            nc.gpsimd.memset(selB[:], 0.0)
                            nc.gpsimd.memset(selB[0:1, 64:128], 1.0)
                            for b in range(2):
                                mask = mask0 if b == 0 else mask1
                                for cch in range(8):
                                    pos = []
                                    rrs = []
                                    for hi in range(2):
                                        h = 2 * cch + hi
                                        half = hi * 64
                                        st4 = apsum.tile([128, 4 * C], F32,
                                                         tag="st4")
                                        for kc in range(4):
                                            k0 = C * b + 128 * kc
                                            ksrc = (kt0[cch][half:half + 64,
                                                             k0:k0 + 128]
                                                    if k0 < 512 else
                                                    kt1[cch][half:half + 64,
                                                             k0 - 512:
                                                             k0 - 384])
                                            nc.tensor.matmul(
                                                st4[:, kc * C:(kc + 1) * C],
                                                _r(ksrc),
                                                _r(qts[b][cch][half:half + 64,
                                                               :]),
                                                start=True, stop=True)
                                        pt4 = ptp.tile([128, 4 * C], BF16,
                                                       tag="pt4")
                                        nc.scalar.activation(
                                            pt4[:], st4[:], AF.Exp,
                                            scale=float(1.0 / np.sqrt(DH)))
                                        meng = nc.vector if hi == 0 \
                                            else nc.gpsimd
                                        meng.tensor_mul(pt4[:], pt4[:],
                                                        mask[:])
                                        po = posum.tile([65, C], F32,
                                                        tag="po")
                                        for kc in range(4):
                                            nc.tensor.matmul(
                                                po[:],
                                                _r(vts[2 * b + kc][:,
                                                   h * 65:(h + 1) * 65]),
                                                _r(pt4[:,
                                                   kc * C:(kc + 1) * C]),
                                                start=(kc == 0),
                                                stop=(kc == 3))
                                        pos.append(po)
                                        rr = rp.tile([1, C], BF16, tag="rr")
                                        with nc.allow_low_precision(
                                                reason="bf16 rounding of "
                                                "softmax denominators is "
                                                "benign"):
                                            nc.vector.reciprocal(
                                                rr[:], po[64:65, :])
                                        rrs.append(rr)
                                    rb2 = rbsum.tile([128, C], F32, tag="rb")
                                    for hi, sel in ((0, selA), (1, selB)):
                                        nc.tensor.matmul(
                                            rb2[:], sel[:], rrs[hi][:],
                                            start=(hi == 0), stop=(hi == 1))
                                    rs2 = rp.tile([128, C], BF16, tag="rs")
                                    nc.scalar.activation(rs2[:], rb2[:],
                                                         AF.Copy, bias=0.0)
                                    for hi in range(2):
                                        half = hi * 64
                                        nc.vector.tensor_mul(
                                            attns[cch][half:half + 64,
                                                       C * b:C * (b + 1)],
                                            pos[hi][0:64, :],
                                            rs2[half:half + 64, :])
                    # act0 closed: k/v/q freed

                    # ---- SSM output + WO + fusion --------------------------
                    act_prefetch(AF.Sqrt)  # LN2 table, during WO window
                    with tc.tile_pool(name="fus", bufs=1) as fus, \
                         tc.tile_pool(name="wfus", bufs=10) as wf, \
                         tc.tile_pool(name="spsum", bufs=4, space="PSUM") as sp:
                        ys = [persist(fus, [128, TOWN], f"y{c}") for c in range(8)]
                        zs = [persist(fus, [128, TOWN], f"z{c}") for c in range(8)]
                        for mc in range(8):
                            ps = sp.tile([128, 512], F32, tag="s")
                            nc.tensor.matmul(
                                ps[:],
                                _r(wc[:, mc * 128:(mc + 1) * 128]),
                                _r(states[:, OWN0:OWN0 + TOWN]),
                                start=True, stop=True)
                            nc.scalar.activation(ys[mc][:], ps[:], AF.Copy,
                                                 bias=0.0)
                            eng = nc.gpsimd if mc % 2 else nc.vector
                            eng.tensor_add(zs[mc][:], ys[mc][:], xos[mc])
                        xms = [persist(xm_pool, [128, TOWN], f"xm{c}")
                               for c in range(8)]
                        half1 = []
                        for mg in range(2):
                            for j in range(4):
                                mc = mg * 4 + j
                                ps = sp.tile([128, 512], F32, tag="s")
                                for kc in range(8):
                                    w0 = kc * 512 + j * 128
                                    nc.tensor.matmul(
                                        ps[:],
                                        _r(wos[mg][:, w0:w0 + 128]),
                                        _r(attns[kc][:]),
                                        start=(kc == 0), stop=(kc == 7))
                                # epilogue split by token halves: half-0 of
                                # every chunk lands first so LN2 stats can
                                # start while half-1 is still in flight
                                xm = xms[mc]

                                def epi(mc, ps, h0, hw):
                                    xm = xms[mc]
                                    nc.vector.scalar_tensor_tensor(
                                        xm[:, h0:h0 + hw], ps[:, h0:h0 + hw],
                                        bias_col(BC_O + mc),
                                        ys[mc][:, h0:h0 + hw],
                                        op0=OP.add, op1=OP.subtract)
                                    eng = (nc.gpsimd if mc in (2, 5, 7)
                                           else nc.vector)
                                    eng.tensor_mul(xm[:, h0:h0 + hw],
                                                   xm[:, h0:h0 + hw],
                                                   gs[mc][:, h0:h0 + hw])
                                    eng.tensor_add(xm[:, h0:h0 + hw],
                                                   xm[:, h0:h0 + hw],
                                                   zs[mc][:, h0:h0 + hw])

                                epi(mc, ps, 0, 256)
                                half1.append((mc, ps))
                        for mc, ps in half1:
                            epi(mc, ps, 256, 256)
                # act1 closed: g/attn/drive/states/xos freed

                # ====== phase group B: LN2 + MLP ==============================
                with tc.tile_pool(name="xn2p", bufs=1) as xn2p, \
                     tc.tile_pool(name="wmlp", bufs=3) as wm, \
                     tc.tile_pool(name="w2p", bufs=1) as w2p:

                    def load_w1g(mg):
                        w = wm.tile([128, 4096], BF16, tag="w1")
                        nc.sync.dma_start(
                            w[:],
                            d_w1[0:1024, mg * 512:mg * 512 + 512].rearrange(
                                "(c p) m -> p c m", p=128))
                        return w

                    with tc.tile_pool(name="lnst2", bufs=1) as lnst2:
                        with tc.tile_pool(name="ln2psum", bufs=2,
                                          space="PSUM") as lnp2:
                            xn2s = layernorm(xms, TOWN, lnst2, lnp2, xn2p, "h",
                                             sq_pool_cs=(1, 4, 6),
                                             norm_pool_cs=(2, 5),
                                             tiles=[(0, 256), (256, 256)])
                        act_prefetch(AF.Gelu)
                    with tc.tile_pool(name="hTp", bufs=1) as hTp:
                        hts = [persist(hTp, [128, TOWN], f"ht{c}")
                               for c in range(32)]
                        with tc.tile_pool(name="m1psum", bufs=6,
                                          space="PSUM") as mp1:
                            for mg in range(8):
                                ws = w1pre if mg == 0 else load_w1g(mg)
                                pss = [mp1.tile([128, 512], F32, tag="m",
                                                name=f"m{j}")
                                       for j in range(4)]
                                for h0 in (0, 256):
                                    for j in range(4):
                                        for kc in range(8):
                                            w0 = kc * 512 + j * 128
                                            nc.tensor.matmul(
                                                pss[j][:, h0:h0 + 256],
                                                _r(ws[:, w0:w0 + 128]),
                                                _r(xn2s[kc][:, h0:h0 + 256]),
                                                start=(kc == 0),
                                                stop=(kc == 7))
                                for j in range(4):
                                    mc = mg * 4 + j
                                    nc.scalar.activation(
                                        hts[mc][:], pss[j][:], AF.Gelu,
                                        bias=bias_col(BC_B1 + mc))
                        # all of W2 resident (64KB/partition, bf16), ONE DMA
                        # issued while MLP1 computes; mc-outer chains below
                        # let each output's epilogue+store overlap the next
                        w2all = persist(w2p, [128, 32 * 1024], "w2all")
                        nc.sync.dma_start(
                            w2all[:],
                            d_w2[0:4096, 0:1024].rearrange(
                                "(c p) m -> p c m", p=128))
                        with tc.tile_pool(name="m2psum", bufs=3,
                                          space="PSUM") as mp2, \
                             tc.tile_pool(name="outp", bufs=3) as outp:
                            for mc in range(8):
                                ps = mp2.tile([128, 512], F32, tag="o")
                                for kc in range(32):
                                    w0 = kc * 1024 + mc * 128
                                    nc.tensor.matmul(
                                        ps[:],
                                        _r(w2all[:, w0:w0 + 128]),
                                        _r(hts[kc][:]),
                                        start=(kc == 0), stop=(kc == 31))
                                oc = outp.tile([128, TOWN], BF16, tag="oc")
                                nc.vector.scalar_tensor_tensor(
                                    oc[:], ps[:], bias_col(BC_B2 + mc),
                                    xms[mc][:], op0=OP.add, op1=OP.add)
                                nc.sync.dma_start(
                                    d_out[mc * 128:(mc + 1) * 128, :], oc[:])

    nc.compile()
    return nc


def _make_masks():
    qi = np.arange(C)[:, None]
    kk = np.arange(2 * C)[None, :]
    band = (kk > qi) & (kk <= qi + C)
    first = band & (kk >= C)

    def pack(m):                       # [C, 2C] -> [128, 4*C] k-chunk-major
        mt = m.T.astype(NPBF16)        # [2C, C]
        return np.ascontiguousarray(
            mt.reshape(4, 128, C).transpose(1, 0, 2).reshape(128, 4 * C))

    return pack(first), pack(band)


def _prep_inputs(x, ln1_g, ln1_b, ln2_g, ln2_b, W_qkv, W_O, b_O, W_ug, b_ug,
                 B_w, A, C_w, mlp_W1, mlp_b1, mlp_W2, mlp_b2):
    f = np.float32
    g1 = np.asarray(ln1_g, f)
    b1 = np.asarray(ln1_b, f)
    W_qkv = np.asarray(W_qkv, f)
    W_qkv_e = g1[:, None] * W_qkv
    b_qkv_e = b1 @ W_qkv
    W_ug = np.asarray(W_ug, f)
    B_w = np.asarray(B_w, f)
    b_ug = np.asarray(b_ug, f)
    W_drive_raw = B_w + W_ug[:, :S]
    W_drive_e = g1[:, None] * W_drive_raw
    b_drive_e = b1 @ W_drive_raw + b_ug[:S]
    W_gate_e = g1[:, None] * W_ug[:, S:]
    b_gate_e = b1 @ W_ug[:, S:] + b_ug[S:]
    g2 = np.asarray(ln2_g, f)
    b2l = np.asarray(ln2_b, f)
    mlp_W1 = np.asarray(mlp_W1, f)
    W1_e = g2[:, None] * mlp_W1
    b1_e = b2l @ mlp_W1 + np.asarray(mlp_b1, f)

    biaspack = np.zeros((128, NBC), f)
    biaspack[:, BC_A] = np.asarray(A, f)
    biaspack[:, BC_QKV:BC_QKV + 24] = b_qkv_e.reshape(24, 128).T
    biaspack[:, BC_GATE:BC_GATE + 8] = b_gate_e.reshape(8, 128).T
    biaspack[:, BC_DRIVE] = b_drive_e
    biaspack[:, BC_O:BC_O + 8] = np.asarray(b_O, f).reshape(8, 128).T
    biaspack[:, BC_B1:BC_B1 + 32] = b1_e.reshape(32, 128).T
    biaspack[:, BC_B2:BC_B2 + 8] = np.asarray(mlp_b2, f).reshape(8, 128).T
    vbias = np.ascontiguousarray(b_qkv_e[2 * D:].reshape(1, D))

    m_first, m_band = _make_masks()
    xTfull = np.ascontiguousarray(np.asarray(x, f)[0].T.astype(NPBF16))

    def bf(a):
        return np.ascontiguousarray(np.asarray(a, f).astype(NPBF16))

    shared = {
        "biaspack": biaspack, "vbias": vbias,
        "wqkv": bf(W_qkv_e),
        "wgate": bf(W_gate_e),
        "wdrive": bf(W_drive_e),
        "wo": bf(W_O),
        "cw": np.ascontiguousarray(np.asarray(C_w, f)),
        "w1": bf(W1_e),
        "w2": bf(mlp_W2),
        "mask1": m_band,
    }
    in_maps = []
    for i in range(NCORES):
        t0 = i * TOWN
        xT = np.zeros((D, TLOC), NPBF16)
        lo = max(0, t0 - HALO)
        xT[:, HALO - (t0 - lo):HALO] = xTfull[:, lo:t0]
        xT[:, HALO:] = xTfull[:, t0:t0 + TOWN]
        m0 = m_first if i == 0 else m_band
        in_maps.append({**shared, "xT": np.ascontiguousarray(xT), "mask0": m0})
    return in_maps


_CACHED_NC = None


def get_nc():
    global _CACHED_NC
    if _CACHED_NC is None:
        _CACHED_NC = build_program()
    return _CACHED_NC


def kernel(**inputs):
    nc = get_nc()
    in_maps = _prep_inputs(**inputs)
    res = bass_utils.run_bass_kernel_spmd(nc, in_maps,
                                          core_ids=list(range(NCORES)))
    out = np.empty((1, T, D), np.float32)
    for i in range(NCORES):
        out[0, i * TOWN:(i + 1) * TOWN, :] = \
            res.results[i]["outT"].astype(np.float32).T
    return out


# revision 56
# speedup vs baseline: 1.4755x; 1.0099x over previous
"""DPA+SSM block kernel for 8 Trainium2 NeuronCores.

Sharding: data-parallel over the sequence (T=4096 -> 8 x 512 own tokens);
each core also receives a 256-token halo of the raw input before its own
range.  The attention window is 256, so the halo covers every key a core
needs; the SSM recurrence decay |A| < 0.1 makes state influence from before
the halo underflow fp32 entirely, so a zero-initialized scan warm-started
over the halo is exact.  No cross-core communication.

Layout: activations are feature-major [D, T] on the device (host transposes
in/out).  Weights and activations are bf16 (halves HBM traffic and doubles
DVE throughput; matmul cost on TRN2 is 1 row/cycle for bf16 and for fp32r
with free dim >= 256, so precision is the only trade — final rel err stays
~7e-3, inside the 2e-2 gate).  LayerNorm statistics use an all-ones
stationary matmul (partition reduction, replicated over partitions), kept
in bf16 with f32 PSUM accumulation.  The SSM scan is one exact f32 DVE
tensor_tensor_scan.  V carries an interleaved ones column per head so each
PV matmul also emits the softmax denominator.

Scheduling notes (from TimelineSim iteration, 379us -> 290us simulated):
 - HWDGE descriptor generation costs a fixed ~625ns per DMA instruction,
   so all loads are batched: one strided (c p) m -> p c m DMA per 8-chunk
   weight group, x in 4 pair-tiles, W2 as a single 8MB DMA (~35 DMAs total
   vs 214 per-chunk).  DMA issue order matches PE consumption order.
 - Tile dependencies are tracked per tile, so tensors are split (x pairs,
   K t0/t1, Q per attention block) to let consumers start early.
 - Projections are emitted in two phases: every chain needing only the
   first LayerNorm t-tile runs before any chain touching the second.
 - Elementwise epilogues are spread across DVE / Pool(gpsimd) / Act so no
   single engine gates the Tensor engine.  HW constraints honored: Pool
   never touches PSUM, DVE reads at most one PSUM operand (softmax
   reciprocals are pair-broadcast via 0/1-selector matmuls and staged to
   SBUF with one Act copy per head pair), matmul operands are never mixed
   32/16-bit, matmul PSUM writes start at partition 0.
 - Act tables (sqrt/sigmoid/exp/gelu live in different sets, 1.28us per
   reload) are prefetched with 1-element dummy ops during idle windows;
   LN squares run as x*x on DVE/Pool to keep the Act queue clear.
 - Fusion epilogue, LN2 and MLP1 are split into token halves so the MLP
   starts on the first half while the second is still normalizing; the
   W2 GEMM runs one output chunk at a time so each epilogue+store
   overlaps the next chunk's accumulation.
"""

import sys

try:
    import concourse.bass as bass  # noqa: F401
except Exception:
    sys.path.insert(0, "/opt/trn_rl_repo")

import numpy as np

import concourse.bass as bass  # noqa: F401
import concourse.mybir as mybir
from concourse import bacc, bass_utils
from concourse.tile import TileContext

F32 = mybir.dt.float32
F32R = mybir.dt.float32r
BF16 = mybir.dt.bfloat16
NPBF16 = mybir.dt.np(mybir.dt.bfloat16)

D = 1024
S = 128
H = 16
DH = 64
C = 256          # attention window / block size
T = 4096
NCORES = 8
TOWN = T // NCORES        # 512 own tokens per core
HALO = C                  # 256 halo tokens
TLOC = TOWN + HALO        # 768 local rows per core
EPS = 1e-5

# bias-pack column layout (f32 per-partition scalars)
BC_A = 0
BC_QKV = 1       # 24 cols
BC_GATE = 25     # 8
BC_DRIVE = 33    # 1
BC_O = 34        # 8
BC_B1 = 42       # 32
BC_B2 = 74       # 8
NBC = 82

T_TILES = [(0, 512), (512, 256)]
OWN0 = HALO


def _r(ap):
    return ap


def build_program(reps=1):
    nc = bacc.Bacc("TRN2", target_bir_lowering=False, debug=False)
    d_xT = nc.dram_tensor("xT", [D, TLOC], BF16, kind="ExternalInput").ap()
    d_mask0 = nc.dram_tensor("mask0", [128, 4 * C], BF16, kind="ExternalInput").ap()
    d_mask1 = nc.dram_tensor("mask1", [128, 4 * C], BF16, kind="ExternalInput").ap()
    d_bias = nc.dram_tensor("biaspack", [128, NBC], F32, kind="ExternalInput").ap()
    d_vbias = nc.dram_tensor("vbias", [1, D], F32R, kind="ExternalInput").ap()
    d_wqkv = nc.dram_tensor("wqkv", [D, 3 * D], BF16, kind="ExternalInput").ap()
    d_wgate = nc.dram_tensor("wgate", [D, D], BF16, kind="ExternalInput").ap()
    d_wdrive = nc.dram_tensor("wdrive", [D, S], BF16, kind="ExternalInput").ap()
    d_wo = nc.dram_tensor("wo", [D, D], BF16, kind="ExternalInput").ap()
    d_cw = nc.dram_tensor("cw", [S, D], F32R, kind="ExternalInput").ap()
    d_w1 = nc.dram_tensor("w1", [D, 4 * D], BF16, kind="ExternalInput").ap()
    d_w2 = nc.dram_tensor("w2", [4 * D, D], BF16, kind="ExternalInput").ap()
    d_out = nc.dram_tensor("outT", [D, TOWN], BF16, kind="ExternalOutput").ap()

    AF = mybir.ActivationFunctionType
    OP = mybir.AluOpType

    def persist(pool, shape, tag, dtype=BF16):
        return pool.tile(shape, dtype, tag=tag, name=tag, bufs=1)

    with TileContext(nc) as tc:
        for _rep in range(reps):
            with tc.tile_pool(name="const", bufs=1) as const, \
                 tc.tile_pool(name="xmid", bufs=1) as xm_pool:
                # const tiles: allocate now; DMAs for bias/masks/vbias are
                # issued AFTER the x-chunk DMAs (those gate the first matmul)
                biasp = persist(const, [128, NBC], "biasp", F32)
                mask0 = persist(const, [128, 4 * C], "mask0")
                mask1 = persist(const, [128, 4 * C], "mask1")
                vbrow = persist(const, [1, D], "vbrow", F32R)
                a_full = persist(const, [128, TLOC], "a_full", F32)
                eps_col = persist(const, [128, 1], "eps_col", F32)
                nc.vector.memset(eps_col[:], EPS)
                dummy = persist(const, [1, 1], "dummy", F32)

                def act_prefetch(func):
                    # 1-element activation to pull the act table in during an
                    # idle window instead of on the critical path
                    nc.scalar.activation(dummy[:], eps_col[0:1, :], func)

                # sqrt table first so the LN1 rstd sqrt needs no reload
                act_prefetch(AF.Sqrt)
                ones_ln = persist(const, [128, 128], "ones_ln", BF16)
                nc.vector.memset(ones_ln[:], 1.0 / D)
                onesr_f = persist(const, [1, 128], "onesr_f", F32)
                nc.vector.memset(onesr_f[:], 1.0)
                ones_row = persist(const, [1, 128], "ones_row", F32R)
                nc.scalar.activation(ones_row[:], onesr_f[:], AF.Copy,
                                     bias=0.0)

                def bias_col(idx):
                    return biasp[:, idx:idx + 1]

                def layernorm(xs, tfree, stats, spsum, out_pool, tagp,
                              sq_pool_cs=(), norm_pool_cs=(2, 6),
                              tiles=None):
                    """Stats via ones-matmul; normalize split DVE/Pool, bf16.

                    Normalization is emitted per (t-tile, chunk) so the first
                    t-tile's outputs exist before the second tile's stats are
                    done — downstream matmul chains start ~5us earlier.
                    """
                    mu = persist(stats, [128, tfree], f"mu{tagp}")
                    rstd = persist(stats, [128, tfree], f"rstd{tagp}")
                    var = persist(stats, [128, tfree], f"var{tagp}", F32)
                    outs = [persist(out_pool, [128, tfree], f"{tagp}{c}")
                            for c in range(8)]
                    ttl = tiles if tiles is not None else \
                        [(t0, tw) for (t0, tw) in T_TILES if t0 < tfree]
                    for t0, tw in ttl:
                        ps_mu = spsum.tile([128, 512], F32, tag="ln_mu")
                        ps_sq = spsum.tile([128, 512], F32, tag="ln_sq")
                        for c in range(8):
                            nc.tensor.matmul(ps_mu[:, :tw], _r(ones_ln[:]),
                                             _r(xs[c][:, t0:t0 + tw]),
                                             start=(c == 0), stop=(c == 7))
                        nc.vector.tensor_copy(mu[:, t0:t0 + tw], ps_mu[:, :tw])
                        for c in range(8):
                            sq = stats.tile([128, 512], BF16, tag="sq", bufs=3)
                            # square on DVE/Pool, not Act: keeps the Act
                            # queue clear of table-set churn
                            seng = (nc.gpsimd if c in sq_pool_cs
                                    else nc.vector)
                            seng.tensor_mul(sq[:, :tw], xs[c][:, t0:t0 + tw],
                                            xs[c][:, t0:t0 + tw])
                            nc.tensor.matmul(ps_sq[:, :tw], _r(ones_ln[:]),
                                             _r(sq[:, :tw]),
                                             start=(c == 0), stop=(c == 7))
                        nc.vector.tensor_mul(var[:, t0:t0 + tw],
                                             mu[:, t0:t0 + tw],
                                             mu[:, t0:t0 + tw])
                        nc.vector.tensor_sub(var[:, t0:t0 + tw],
                                             ps_sq[:, :tw], var[:, t0:t0 + tw])
                        nc.scalar.activation(var[:, t0:t0 + tw],
                                             var[:, t0:t0 + tw], AF.Sqrt,
                                             bias=eps_col[:])
                        with nc.allow_low_precision(
                                reason="bf16 rstd: 0.4% scale error on "
                                "normalized activations is benign"):
                            nc.vector.reciprocal(rstd[:, t0:t0 + tw],
                                                 var[:, t0:t0 + tw])
                        for c in range(8):
                            o = outs[c]
                            eng = nc.gpsimd if c in norm_pool_cs else nc.vector
                            eng.tensor_sub(o[:, t0:t0 + tw],
                                           xs[c][:, t0:t0 + tw],
                                           mu[:, t0:t0 + tw])
                            eng.tensor_mul(o[:, t0:t0 + tw],
                                           o[:, t0:t0 + tw],
                                           rstd[:, t0:t0 + tw])
                    return outs

                # ====== phase group A: LN1, projections, attention, fusion ====
                with tc.tile_pool(name="act1", bufs=1) as act1:
                    gs = [persist(act1, [128, TOWN], f"g{c}") for c in range(8)]
                    attns = [persist(act1, [128, TOWN], f"at{c}")
                             for c in range(8)]
                    driveT = persist(act1, [128, TLOC], "driveT", F32)
                    states = persist(act1, [128, TLOC], "states", F32R)
                    # x resident for the whole phase: LN1 input + residual.
                    # One wide tile, TWO DMAs total (HWDGE descriptor
                    # generation is a fixed 625ns per DMA instruction, so
                    # fewer/wider transfers beat per-chunk loads)
                    wc = persist(act1, [128, 1024], "wc", F32R)
                    wos = [persist(act1, [128, 4096], f"wo{mg}")
                           for mg in range(2)]
                    xpairs = [persist(act1, [128, 2 * TLOC], f"xp{i}")
                              for i in range(4)]
                    xts = [xpairs[c // 2][:, (c % 2) * TLOC:
                                          (c % 2 + 1) * TLOC]
                           for c in range(8)]
                    xos = [xpairs[c // 2][:, (c % 2) * TLOC + OWN0:
                                          (c % 2 + 1) * TLOC]
                           for c in range(8)]

                    with tc.tile_pool(name="act0", bufs=1) as act0:
                        kt0 = [persist(act0, [128, 512], f"k0_{c}")
                               for c in range(8)]
                        kt1 = [persist(act0, [128, 256], f"k1_{c}")
                               for c in range(8)]
                        vts = [persist(act0, [128, 16 * 65], f"v{c}")
                               for c in range(6)]
                        qts = [[persist(act0, [128, 256], f"q{hb}_{c}")
                                for c in range(8)] for hb in range(2)]
                        for tt in range(6):
                            vr = vts[tt][:].rearrange("p (h e) -> p h e", e=65)
                            nc.gpsimd.memset(vr[:, :, 64:65], 1.0)

                        with tc.tile_pool(name="xnp", bufs=1) as xnp, \
                             tc.tile_pool(name="lnst", bufs=1) as lnst:
                            for i in range(4):
                                nc.sync.dma_start(
                                    xpairs[i][:],
                                    d_xT[i * 256:(i + 1) * 256, :].rearrange(
                                        "(c p) t -> p c t", p=128))
                            with tc.tile_pool(name="lnpsum", bufs=2,
                                              space="PSUM") as lnp:
                                xns = layernorm(xts, TLOC, lnst, lnp, xnp, "xn")

                            # ---- projections -------------------------------
                            # one DMA per 8-chunk weight group: HWDGE costs a
                            # fixed 625ns per DMA instruction, so group loads
                            # through a strided (c p) m -> p (c m) pattern
                            with tc.tile_pool(name="wlin", bufs=8) as wp, \
                                 tc.tile_pool(name="linpsum", bufs=5,
                                              space="PSUM") as psum:

                                def load_wg(dram, m0, mw, tag="w", bufs=None):
                                    w = wp.tile([128, 8 * mw], BF16, tag=tag,
                                                bufs=bufs)
                                    nc.sync.dma_start(
                                        w[:],
                                        dram[0:1024, m0:m0 + mw].rearrange(
                                            "(c p) m -> p c m", p=128))
                                    return w

                                # All chains split by LN1 t-tile: every chain
                                # that only needs tile-0 xn (cols 0:512) is
                                # emitted before any chain touching tile 1,
                                # so the in-order PE queue streams behind the
                                # LN1 normalize instead of stalling on it.
                                # load order matches PE consumption order
                                # in the t0 phase: gate -> Q -> K -> V
                                wgates = [load_wg(d_wgate, mg * 512, 512)
                                          for mg in range(2)]
                                nc.sync.dma_start(biasp[:], d_bias[:])
                                wqs = [load_wg(d_wqkv, mg * 512, 512)
                                       for mg in range(2)]
                                wks = [load_wg(d_wqkv, D + mg * 512, 512)
                                       for mg in range(2)]
                                nc.sync.dma_start(vbrow[:], d_vbias[:])
                                wvs = [load_wg(d_wqkv, 2 * D + vc * 512, 512)
                                       for vc in range(2)]
                                wd = load_wg(d_wdrive, 0, 128, tag="wd",
                                             bufs=1)

                                def gate_q(t0, tw):
                                    # gate first: its sigmoids are the only
                                    # Act work here, so Act reaches the
                                    # exp-table prefetch early
                                    for wgrp, outs, bc0, act in (
                                            (wgates, gs, BC_GATE, True),
                                            (wqs, qts, BC_QKV, False)):
                                        for mg in range(2):
                                            for j in range(4):
                                                mc = mg * 4 + j
                                                ps = psum.tile([128, 512], F32,
                                                               tag="lin")
                                                for kc in range(8):
                                                    w0 = kc * 512 + j * 128
                                                    nc.tensor.matmul(
                                                        ps[:, :tw],
                                                        _r(wgrp[mg][:,
                                                           w0:w0 + 128]),
                                                        _r(xns[kc][:,
                                                           t0:t0 + tw]),
                                                        start=(kc == 0),
                                                        stop=(kc == 7))
                                                o0 = t0 - OWN0
                                                if act:
                                                    nc.scalar.activation(
                                                        outs[mc][:, o0:o0 + tw],
                                                        ps[:, :tw], AF.Sigmoid,
                                                        bias=bias_col(bc0 + mc))
                                                else:
                                                    qdst = outs[0 if o0 == 0
                                                                else 1][mc]
                                                    nc.vector.tensor_scalar(
                                                        qdst[:, :tw],
                                                        ps[:, :tw],
                                                        bias_col(bc0 + mc),
                                                        None, OP.add)

                                def k_drive(t0, tw):
                                    for mg in range(2):
                                        for j in range(4):
                                            mc = mg * 4 + j
                                            ps = psum.tile([128, 512], F32,
                                                           tag="lin")
                                            for kc in range(8):
                                                w0 = kc * 512 + j * 128
                                                nc.tensor.matmul(
                                                    ps[:, :tw],
                                                    _r(wks[mg][:, w0:w0 + 128]),
                                                    _r(xns[kc][:, t0:t0 + tw]),
                                                    start=(kc == 0),
                                                    stop=(kc == 7))
                                            kdst = (kt0[mc][:, t0:t0 + tw]
                                                    if t0 == 0 else
                                                    kt1[mc][:, :tw])
                                            nc.vector.tensor_scalar(
                                                kdst, ps[:, :tw],
                                                bias_col(BC_QKV + 8 + mc),
                                                None, OP.add)
                                    ps = psum.tile([128, 512], F32, tag="lin")
                                    for kc in range(8):
                                        nc.tensor.matmul(
                                            ps[:, :tw],
                                            _r(wd[:, kc * 128:(kc + 1) * 128]),
                                            _r(xns[kc][:, t0:t0 + tw]),
                                            start=(kc == 0), stop=(kc == 7))
                                    nc.vector.tensor_scalar(
                                        driveT[:, t0:t0 + tw], ps[:, :tw],
                                        bias_col(BC_DRIVE), None, OP.add)

                                def v_block(tts):
                                    # V bias folded into the accumulation as
                                    # a rank-1 ones x vbrow matmul
                                    for vc in range(2):
                                        for tt in tts:
                                            t0 = tt * 128
                                            ps = psum.tile([128, 512], F32,
                                                           tag="lin")
                                            for kc in range(8):
                                                nc.tensor.matmul(
                                                    ps[:],
                                                    _r(xns[kc][:, t0:t0 + 128]),
                                                    _r(wvs[vc][:, kc * 512:
                                                               (kc + 1) * 512]),
                                                    start=(kc == 0), stop=False)
                                            nc.tensor.matmul(
                                                ps[:], ones_row[:, 0:128],
                                                vbrow[:,
                                                      vc * 512:(vc + 1) * 512],
                                                start=False, stop=True)
                                            vr = vts[tt][:].rearrange(
                                                "p (h e) -> p h e", e=65)
                                            nc.scalar.activation(
                                                vr[:, vc * 8:(vc + 1) * 8,
                                                   0:64],
                                                ps[:].rearrange(
                                                    "p (h e) -> p h e", e=64),
                                                AF.Copy, bias=0.0)

                                # tile-0-only chains
                                gate_q(OWN0, 512 - OWN0)
                                k_drive(0, 512)
                                v_block(range(4))
                                # tile-1 chains
                                gate_q(512, TLOC - 512)
                                act_prefetch(AF.Exp)
                                k_drive(512, TLOC - 512)
                                v_block((4, 5))
                        # xnp closed: xn freed before attention

                        # masks overlap late projections (x stays resident,
                        # no residual re-load needed)
                        nc.sync.dma_start(mask0[:], d_mask0[:])
                        nc.sync.dma_start(mask1[:], d_mask1[:])
                        # fusion + first MLP weights stream during attention
                        nc.sync.dma_start(wc[:], d_cw[:])
                        for mg in range(2):
                            nc.sync.dma_start(
                                wos[mg][:],
                                d_wo[0:1024,
                                     mg * 512:mg * 512 + 512].rearrange(
                                    "(c p) m -> p c m", p=128))
                        w1pre = persist(xm_pool, [128, 4096], "w1g0")
                        nc.sync.dma_start(
                            w1pre[:],
                            d_w1[0:1024, 0:512].rearrange(
                                "(c p) m -> p c m", p=128))
                        # scan coefficient, off the critical path on Pool
                        nc.gpsimd.memset(a_full[:], 1.0)
                        nc.gpsimd.tensor_scalar_mul(a_full[:], a_full[:],
                                                    biasp[:, BC_A:BC_A + 1])
                        # SSM scan emitted BEFORE attention: driveT is ready,
                        # and queueing it here keeps it off the back of the
                        # attention-heavy DVE queue, so states exist the
                        # moment the WO/Cw matmuls want them
                        nc.vector.tensor_tensor_scan(states[:], a_full[:],
                                                     driveT[:], 0.0,
                                                     OP.mult, OP.add)

                        # ---- windowed attention ----------------------------
                        # processed per head PAIR (both heads of one cch):
                        # one [2,128] selector matmul broadcasts both heads'
                        # softmax reciprocals at once, one Act copy stages
                        # them in SBUF (HW: DVE may read only one PSUM
                        # operand, Pool none)
                        with tc.tile_pool(name="apsum", bufs=2,
                                          space="PSUM") as apsum, \
                             tc.tile_pool(name="posum", bufs=3,
                                          space="PSUM") as posum, \
                             tc.tile_pool(name="rbsum", bufs=1,
                                          space="PSUM") as rbsum, \
                             tc.tile_pool(name="ptp", bufs=3) as ptp, \
                             tc.tile_pool(name="rp", bufs=4) as rp:
                            # 0/1 selector rows: rb2 = selA^T@rrA + selB^T@rrB
                            # broadcasts both heads' reciprocals in one
                            # [128,C] accumulation with a partition-0 dst
                            selA = persist(rp, [1, 128], "selA")
                            nc.gpsimd.memset(selA[:], 0.0)
                            nc.gpsimd.memset(selA[0:1, 0:64], 1.0)
                            selB = persist(rp, [1, 128], "selB")
                            nc.gpsimd.memset(selB[:], 0.0)
                            nc.gpsimd.memset(selB[0:1, 64:128], 1.0)
                            for b in range(2):
                                mask = mask0 if b == 0 else mask1
                                for cch in range(8):
                                    pos = []
                                    rrs = []
                                    for hi in range(2):
                                        h = 2 * cch + hi
                                        half = hi * 64
                                        st4 = apsum.tile([128, 4 * C], F32,
                                                         tag="st4")
                                        for kc in range(4):
                                            k0 = C * b + 128 * kc
                                            ksrc = (kt0[cch][half:half + 64,
                                                             k0:k0 + 128]
                                                    if k0 < 512 else
                                                    kt1[cch][half:half + 64,
                                                             k0 - 512:
                                                             k0 - 384])
                                            nc.tensor.matmul(
                                                st4[:, kc * C:(kc + 1) * C],
                                                _r(ksrc),
                                                _r(qts[b][cch][half:half + 64,
                                                               :]),
                                                start=True, stop=True)
                                        pt4 = ptp.tile([128, 4 * C], BF16,
                                                       tag="pt4")
                                        nc.scalar.activation(
                                            pt4[:], st4[:], AF.Exp,
                                            scale=float(1.0 / np.sqrt(DH)))
                                        meng = nc.vector if hi == 0 \
                                            else nc.gpsimd
                                        meng.tensor_mul(pt4[:], pt4[:],
                                                        mask[:])
                                        po = posum.tile([65, C], F32,
                                                        tag="po")
                                        for kc in range(4):
                                            nc.tensor.matmul(
                                                po[:],
                                                _r(vts[2 * b + kc][:,
                                                   h * 65:(h + 1) * 65]),
                                                _r(pt4[:,
                                                   kc * C:(kc + 1) * C]),
                                                start=(kc == 0),
                                                stop=(kc == 3))
                                        pos.append(po)
                                        rr = rp.tile([1, C], BF16, tag="rr")
                                        with nc.allow_low_precision(
                                                reason="bf16 rounding of "
                                                "softmax denominators is "
                                                "benign"):
                                            nc.vector.reciprocal(
                                                rr[:], po[64:65, :])
                                        rrs.append(rr)
                                    rb2 = rbsum.tile([128, C], F32, tag="rb")
                                    for hi, sel in ((0, selA), (1, selB)):
                                        nc.tensor.matmul(
                                            rb2[:], sel[:], rrs[hi][:],
                                            start=(hi == 0), stop=(hi == 1))
                                    rs2 = rp.tile([128, C], BF16, tag="rs")
                                    nc.scalar.activation(rs2[:], rb2[:],
                                                         AF.Copy, bias=0.0)
                                    for hi in range(2):
                                        half = hi * 64
                                        nc.vector.tensor_mul(
                                            attns[cch][half:half + 64,
                                                       C * b:C * (b + 1)],
                                            pos[hi][0:64, :],
                                            rs2[half:half + 64, :])
                    # act0 closed: k/v/q freed

                    # ---- SSM output + WO + fusion --------------------------
                    act_prefetch(AF.Sqrt)  # LN2 table, during WO window
                    with tc.tile_pool(name="fus", bufs=1) as fus, \
                         tc.tile_pool(name="wfus", bufs=10) as wf, \
                         tc.tile_pool(name="spsum", bufs=4, space="PSUM") as sp:
                        ys = [persist(fus, [128, TOWN], f"y{c}") for c in range(8)]
                        zs = [persist(fus, [128, TOWN], f"z{c}") for c in range(8)]
                        for mc in range(8):
                            ps = sp.tile([128, 512], F32, tag="s")
                            nc.tensor.matmul(
                                ps[:],
                                _r(wc[:, mc * 128:(mc + 1) * 128]),
                                _r(states[:, OWN0:OWN0 + TOWN]),
                                start=True, stop=True)
                            nc.scalar.activation(ys[mc][:], ps[:], AF.Copy,
                                                 bias=0.0)
                            eng = nc.gpsimd if mc % 2 else nc.vector
                            eng.tensor_add(zs[mc][:], ys[mc][:], xos[mc])
                        xms = [persist(xm_pool, [128, TOWN], f"xm{c}")
                               for c in range(8)]
                        half1 = []
                        for mg in range(2):
                            for j in range(4):
                                mc = mg * 4 + j
                                ps = sp.tile([128, 512], F32, tag="s")
                                for kc in range(8):
                                    w0 = kc * 512 + j * 128
                                    nc.tensor.matmul(
                                        ps[:],
                                        _r(wos[mg][:, w0:w0 + 128]),
                                        _r(attns[kc][:]),
                                        start=(kc == 0), stop=(kc == 7))
                                # epilogue split by token halves: half-0 of
                                # every chunk lands first so LN2 stats can
                                # start while half-1 is still in flight
                                xm = xms[mc]

                                def epi(mc, ps, h0, hw):
                                    xm = xms[mc]
                                    nc.vector.scalar_tensor_tensor(
                                        xm[:, h0:h0 + hw], ps[:, h0:h0 + hw],
                                        bias_col(BC_O + mc),
                                        ys[mc][:, h0:h0 + hw],
                                        op0=OP.add, op1=OP.subtract)
                                    eng = (nc.gpsimd if mc in (2, 5, 7)
                                           else nc.vector)
                                    eng.tensor_mul(xm[:, h0:h0 + hw],
                                                   xm[:, h0:h0 + hw],
                                                   gs[mc][:, h0:h0 + hw])
                                    eng.tensor_add(xm[:, h0:h0 + hw],
                                                   xm[:, h0:h0 + hw],
                                                   zs[mc][:, h0:h0 + hw])

                                epi(mc, ps, 0, 256)
                                half1.append((mc, ps))
                        for mc, ps in half1:
                            epi(mc, ps, 256, 256)
                # act1 closed: g/attn/drive/states/xos freed

                # ====== phase group B: LN2 + MLP ==============================
                with tc.tile_pool(name="xn2p", bufs=1) as xn2p, \
                     tc.tile_pool(name="wmlp", bufs=3) as wm, \
                     tc.tile_pool(name="w2p", bufs=1) as w2p:

                    def load_w1g(mg):
                        w = wm.tile([128, 4096], BF16, tag="w1")
                        nc.sync.dma_start(
                            w[:],
                            d_w1[0:1024, mg * 512:mg * 512 + 512].rearrange(
                                "(c p) m -> p c m", p=128))
                        return w

                    with tc.tile_pool(name="lnst2", bufs=1) as lnst2:
                        with tc.tile_pool(name="ln2psum", bufs=2,
                                          space="PSUM") as lnp2:
                            xn2s = layernorm(xms, TOWN, lnst2, lnp2, xn2p, "h",
                                             sq_pool_cs=(1, 4, 6),
                                             norm_pool_cs=(),
                                             tiles=[(0, 256), (256, 256)])
                        act_prefetch(AF.Gelu)
                    with tc.tile_pool(name="hTp", bufs=1) as hTp:
                        hts = [persist(hTp, [128, TOWN], f"ht{c}")
                               for c in range(32)]
                        with tc.tile_pool(name="m1psum", bufs=6,
                                          space="PSUM") as mp1:
                            for mg in range(8):
                                ws = w1pre if mg == 0 else load_w1g(mg)
                                pss = [mp1.tile([128, 512], F32, tag="m",
                                                name=f"m{j}")
                                       for j in range(4)]
                                for h0 in (0, 256):
                                    for j in range(4):
                                        for kc in range(8):
                                            w0 = kc * 512 + j * 128
                                            nc.tensor.matmul(
                                                pss[j][:, h0:h0 + 256],
                                                _r(ws[:, w0:w0 + 128]),
                                                _r(xn2s[kc][:, h0:h0 + 256]),
                                                start=(kc == 0),
                                                stop=(kc == 7))
                                for j in range(4):
                                    mc = mg * 4 + j
                                    nc.scalar.activation(
                                        hts[mc][:], pss[j][:], AF.Gelu,
                                        bias=bias_col(BC_B1 + mc))
                        # all of W2 resident (64KB/partition, bf16), ONE DMA
                        # issued while MLP1 computes; mc-outer chains below
                        # let each output's epilogue+store overlap the next
                        w2all = persist(w2p, [128, 32 * 1024], "w2all")
                        nc.sync.dma_start(
                            w2all[:],
                            d_w2[0:4096, 0:1024].rearrange(
                                "(c p) m -> p c m", p=128))
                        with tc.tile_pool(name="m2psum", bufs=3,
                                          space="PSUM") as mp2, \
                             tc.tile_pool(name="outp", bufs=3) as outp:
                            for mc in range(8):
                                ps = mp2.tile([128, 512], F32, tag="o")
                                for kc in range(32):
                                    w0 = kc * 1024 + mc * 128
                                    nc.tensor.matmul(
                                        ps[:],
                                        _r(w2all[:, w0:w0 + 128]),
                                        _r(hts[kc][:]),
                                        start=(kc == 0), stop=(kc == 31))
                                oc = outp.tile([128, TOWN], BF16, tag="oc")
                                nc.vector.scalar_tensor_tensor(
                                    oc[:], ps[:], bias_col(BC_B2 + mc),
                                    xms[mc][:], op0=OP.add, op1=OP.add)
                                nc.sync.dma_start(
                                    d_out[mc * 128:(mc + 1) * 128, :], oc[:])

    nc.compile()
    return nc


def _make_masks():
    qi = np.arange(C)[:, None]
    kk = np.arange(2 * C)[None, :]
    band = (kk > qi) & (kk <= qi + C)
    first = band & (kk >= C)

    def pack(m):                       # [C, 2C] -> [128, 4*C] k-chunk-major
        mt = m.T.astype(NPBF16)        # [2C, C]
        return np.ascontiguousarray(
            mt.reshape(4, 128, C).transpose(1, 0, 2).reshape(128, 4 * C))

    return pack(first), pack(band)


def _prep_inputs(x, ln1_g, ln1_b, ln2_g, ln2_b, W_qkv, W_O, b_O, W_ug, b_ug,
                 B_w, A, C_w, mlp_W1, mlp_b1, mlp_W2, mlp_b2):
    f = np.float32
    g1 = np.asarray(ln1_g, f)
    b1 = np.asarray(ln1_b, f)
    W_qkv = np.asarray(W_qkv, f)
    W_qkv_e = g1[:, None] * W_qkv
    b_qkv_e = b1 @ W_qkv
    W_ug = np.asarray(W_ug, f)
    B_w = np.asarray(B_w, f)
    b_ug = np.asarray(b_ug, f)
    W_drive_raw = B_w + W_ug[:, :S]
    W_drive_e = g1[:, None] * W_drive_raw
    b_drive_e = b1 @ W_drive_raw + b_ug[:S]
    W_gate_e = g1[:, None] * W_ug[:, S:]
    b_gate_e = b1 @ W_ug[:, S:] + b_ug[S:]
    g2 = np.asarray(ln2_g, f)
    b2l = np.asarray(ln2_b, f)
    mlp_W1 = np.asarray(mlp_W1, f)
    W1_e = g2[:, None] * mlp_W1
    b1_e = b2l @ mlp_W1 + np.asarray(mlp_b1, f)

    biaspack = np.zeros((128, NBC), f)
    biaspack[:, BC_A] = np.asarray(A, f)
    biaspack[:, BC_QKV:BC_QKV + 24] = b_qkv_e.reshape(24, 128).T
    biaspack[:, BC_GATE:BC_GATE + 8] = b_gate_e.reshape(8, 128).T
    biaspack[:, BC_DRIVE] = b_drive_e
    biaspack[:, BC_O:BC_O + 8] = np.asarray(b_O, f).reshape(8, 128).T
    biaspack[:, BC_B1:BC_B1 + 32] = b1_e.reshape(32, 128).T
    biaspack[:, BC_B2:BC_B2 + 8] = np.asarray(mlp_b2, f).reshape(8, 128).T
    vbias = np.ascontiguousarray(b_qkv_e[2 * D:].reshape(1, D))

    m_first, m_band = _make_masks()
    xTfull = np.ascontiguousarray(np.asarray(x, f)[0].T.astype(NPBF16))

    def bf(a):
        return np.ascontiguousarray(np.asarray(a, f).astype(NPBF16))

    shared = {
        "biaspack": biaspack, "vbias": vbias,
        "wqkv": bf(W_qkv_e),
        "wgate": bf(W_gate_e),
        "wdrive": bf(W_drive_e),
        "wo": bf(W_O),
        "cw": np.ascontiguousarray(np.asarray(C_w, f)),
        "w1": bf(W1_e),
        "w2": bf(mlp_W2),
        "mask1": m_band,
    }
    in_maps = []
    for i in range(NCORES):
        t0 = i * TOWN
        xT = np.zeros((D, TLOC), NPBF16)
        lo = max(0, t0 - HALO)
        xT[:, HALO - (t0 - lo):HALO] = xTfull[:, lo:t0]
        xT[:, HALO:] = xTfull[:, t0:t0 + TOWN]
        m0 = m_first if i == 0 else m_band
        in_maps.append({**shared, "xT": np.ascontiguousarray(xT), "mask0": m0})
    return in_maps


_CACHED_NC = None


def get_nc():
    global _CACHED_NC
    if _CACHED_NC is None:
        _CACHED_NC = build_program()
    return _CACHED_NC


def kernel(**inputs):
    nc = get_nc()
    in_maps = _prep_inputs(**inputs)
    res = bass_utils.run_bass_kernel_spmd(nc, in_maps,
                                          core_ids=list(range(NCORES)))
    out = np.empty((1, T, D), np.float32)
    for i in range(NCORES):
        out[0, i * TOWN:(i + 1) * TOWN, :] = \
            res.results[i]["outT"].astype(np.float32).T
    return out
